# revision 7
# baseline (speedup 1.0000x reference)
"""DynamicDecayMemory Trainium2 kernel.

Full inputs: memory (16,256,256), keys (16,4096,256), values (16,4096,256).
Data-parallel over batch: 8 cores x 2 batches each. The sequential scan is
reformulated as chunked (C=128) triangular solves in "w-space"
(u_t = P_t * w_t, P = cumprod(1-d)) solved by Neumann iteration with the
kn-Gram matrix; decay d_t recovered via a small fixed point. The global
cross-batch max of surprise norms is handled with two launches: pass 1 uses
the local 2-batch max and outputs per-step surprise norms; the host reduces
the global per-step max; pass 2 consumes it (validated ~1e-5 rel err).
"""
import sys
import numpy as np

sys.path.insert(0, "/opt/trn_rl_repo")

import concourse.bass as bass
import concourse.bacc as bacc
import concourse.mybir as mybir
import concourse.tile as tile
from concourse import masks
from concourse.bass_utils import run_bass_kernel_spmd
from contextlib import ExitStack

F32 = mybir.dt.float32
F32R = mybir.dt.float32r
AL = mybir.AluOpType
AF = mybir.ActivationFunctionType

B_LOC = 2
S = 4096
C = 128
NCH = S // C
DK = 256
DV = 256
EPS = 1e-6
MAXN_EPS = 256.0 + EPS
D0 = 0.0108

_cache = {}


def _emit(nc, use_mhat):
    keys_d = nc.dram_tensor("keys", [B_LOC, S, DK], F32, kind="ExternalInput")
    vals_d = nc.dram_tensor("vals", [B_LOC, S, DV], F32, kind="ExternalInput")
    mem_d = nc.dram_tensor("mem", [B_LOC, DV, DK], F32, kind="ExternalInput")
    n2in_d = nc.dram_tensor("n2in", [B_LOC, 1], F32, kind="ExternalInput")
    mhat_d = nc.dram_tensor("mhat", [1, S], F32, kind="ExternalInput")
    out_d = nc.dram_tensor("out", [B_LOC, DV, DK], F32, kind="ExternalOutput")
    nrm_d = nc.dram_tensor("nrm", [B_LOC, C, NCH], F32, kind="ExternalOutput")

    NSOLVE = 3 if use_mhat else 2
    NIT = [4, 3, 3]

    with tile.TileContext(nc) as tc, ExitStack() as ctx:
        per = ctx.enter_context(tc.tile_pool(name="per", bufs=1))
        wk = ctx.enter_context(tc.tile_pool(name="wk", bufs=2))
        ps = ctx.enter_context(tc.tile_pool(name="ps", bufs=1, space="PSUM"))
        ps2 = ctx.enter_context(tc.tile_pool(name="ps2", bufs=2, space="PSUM"))

        KnN = [per.tile([C, NCH * DK], F32, tag=f"kn{b}", name=f"kn{b}") for b in range(B_LOC)]
        V = [per.tile([C, NCH * DV], F32, tag=f"v{b}", name=f"v{b}") for b in range(B_LOC)]
        MT = [[per.tile([128, DV], F32, tag=f"mt{b}{i}", name=f"mt{b}{i}") for i in range(2)]
              for b in range(B_LOC)]
        knsq = [per.tile([C, NCH], F32, tag=f"ksq{b}", name=f"ksq{b}") for b in range(B_LOC)]
        v2 = [per.tile([C, NCH], F32, tag=f"v2{b}", name=f"v2{b}") for b in range(B_LOC)]
        snall = [per.tile([C, NCH], F32, tag=f"sna{b}", name=f"sna{b}") for b in range(B_LOC)]

        ident = per.tile([128, 128], F32, tag="ident", name="ident")
        masks.make_identity(nc, ident[:])
        maskUneg = per.tile([128, 128], F32, tag="msku", name="msku")
        masks.make_upper_triangular(nc, maskUneg[:], val=-1.0, diag=False)
        sel127 = per.tile([128, 128], F32, tag="sel127", name="sel127")
        nc.gpsimd.memset(sel127[:], 0.0)
        nc.gpsimd.affine_select(out=sel127[:], in_=sel127[:],
                                compare_op=AL.not_equal, fill=1.0, base=-127,
                                pattern=[[0, 128]], channel_multiplier=1)
        # absorber: a PE op depending only on Pool-made tiles, so later fp32
        # matmuls need just one (DVE) wait.
        absps = ps2.tile([128, 128], F32, tag="tp", name="absps")
        nc.tensor.transpose(absps[:], ident[:], ident[:])

        zeros2 = per.tile([8, C], F32, tag="zr", name="zr")
        nc.vector.memset(zeros2[:], 0.0)
        mhat_t = per.tile([1, S], F32, tag="mhat", name="mhat")
        if use_mhat:
            nc.sync.dma_start(mhat_t[:], mhat_d[:])
        n2in_t = per.tile([B_LOC, 1], F32, tag="n2in", name="n2in")
        nc.sync.dma_start(n2in_t[:], n2in_d[:])

        # constant-d columns: P0/Pm10 as [128,1] cols via row-scan + transpose
        d0row = per.tile([2, 3 * C], F32, tag="d0r", name="d0r")
        nc.vector.memset(d0row[:, 0:C], 1.0 - D0)
        nc.vector.tensor_tensor_scan(d0row[:, C:2 * C], d0row[:, 0:C],
                                     zeros2[0:2, :], 1.0, op0=AL.mult, op1=AL.add)
        nc.vector.memset(d0row[:, 2 * C:2 * C + 1], 1.0)
        nc.vector.tensor_copy(d0row[:, 2 * C + 1:3 * C], d0row[:, C:2 * C - 1])
        pk_ps = ps.tile([128, 8], F32, tag="sm", name="pk")
        nc.tensor.transpose(pk_ps[:, 0:2], d0row[0:2, C:2 * C], ident[0:2, 0:2])
        nc.tensor.transpose(pk_ps[:, 2:4], d0row[0:2, 2 * C:3 * C], ident[0:2, 0:2])
        cstPP = per.tile([128, 2], F32, tag="cstpp", name="cstpp")
        nc.vector.tensor_copy(cstPP[:, 0:1], pk_ps[:, 0:1])   # P0 col
        nc.vector.tensor_copy(cstPP[:, 1:2], pk_ps[:, 2:3])   # Pm10 col
        rPm10 = per.tile([128, 1], F32, tag="rpm0", name="rpm0")
        nc.vector.reciprocal(rPm10[:], cstPP[:, 1:2])
        g1c = 1.1 / (1.0 - D0)

        N2tiles = [per.tile([2, C], F32, tag=f"n2_{i}", name=f"n2_{i}") for i in range(4)]
        carry_ap = n2in_t[:]

        for c in range(NCH):
            c0 = c * C
            KT = [[wk.tile([128, C], F32, tag=f"kt{b}{i}", name=f"kt{b}{i}") for i in range(2)]
                  for b in range(B_LOC)]
            Gsn = [wk.tile([128, C], F32R, tag=f"g{b}", name=f"g{b}") for b in range(B_LOC)]
            A = [wk.tile([C, DV], F32, tag=f"a{b}", name=f"a{b}") for b in range(B_LOC)]
            W = [wk.tile([C, DV], F32R, tag=f"w{b}", name=f"w{b}") for b in range(B_LOC)]
            R1 = [wk.tile([C, DV], F32, tag=f"r1{b}", name=f"r1{b}") for b in range(B_LOC)]
            etile = [wk.tile([C, DV], F32, tag=f"e{b}", name=f"e{b}") for b in range(B_LOC)]
            utile = [wk.tile([C, DV], F32, tag=f"u{b}", name=f"u{b}") for b in range(B_LOC)]
            sjunk = wk.tile([C, DV], F32, tag="sj", name="sj")
            colsG = [wk.tile([128, 8], F32, tag=f"cg{b}", name=f"cg{b}") for b in range(B_LOC)]
            COLP = wk.tile([128, 6], F32, tag="colp", name="colp")
            ROWP = wk.tile([2, 3 * C], F32, tag="rowp", name="rowp")
            ROWP2 = wk.tile([2, 3 * C], F32, tag="rowp2", name="rowp2")
            COL2 = wk.tile([128, 6], F32, tag="col2", name="col2")

            for b in range(B_LOC):
                KNc = KnN[b][:, c * DK:(c + 1) * DK]
                Vc = V[b][:, c * DV:(c + 1) * DV]
                ktmp = wk.tile([C, DK], F32, tag=f"ktmp{b}", name=f"ktmp{b}")
                nc.sync.dma_start(ktmp[:], keys_d[b, c0:c0 + C, :])
                nc.sync.dma_start(Vc, vals_d[b, c0:c0 + C, :])
                nrm2 = wk.tile([C, 1], F32, tag=f"nn{b}", name=f"nn{b}")
                nc.scalar.activation(sjunk[:], ktmp[:], AF.Square, accum_out=nrm2[:])
                nrm = wk.tile([C, 1], F32, tag=f"nr{b}", name=f"nr{b}")
                nc.scalar.sqrt(nrm[:], nrm2[:])
                nrme = wk.tile([C, 1], F32, tag=f"ne{b}", name=f"ne{b}")
                nc.vector.tensor_scalar_add(nrme[:], nrm[:], EPS)
                rk = wk.tile([C, 1], F32, tag=f"rk{b}", name=f"rk{b}")
                nc.vector.reciprocal(rk[:], nrme[:])
                nc.vector.tensor_scalar_mul(KNc, ktmp[:], rk[:])
                t0 = wk.tile([C, 1], F32, tag=f"t0{b}", name=f"t0{b}")
                nc.vector.tensor_tensor(t0[:], nrm[:], rk[:], op=AL.mult)
                nc.vector.tensor_tensor(knsq[b][:, c:c + 1], t0[:], t0[:], op=AL.mult)
                nc.scalar.activation(sjunk[:], Vc, AF.Square,
                                     accum_out=v2[b][:, c:c + 1])
                if c == 0:
                    for i in range(2):
                        mnat = wk.tile([128, DK], F32, tag=f"mn{b}", name=f"mn{b}")
                        nc.sync.dma_start(mnat[:], mem_d[b, i * 128:(i + 1) * 128, :])
                        for k in range(2):
                            tp = ps2.tile([128, 128], F32, tag="tp", name="tp")
                            nc.tensor.transpose(tp[:], mnat[:, k * 128:(k + 1) * 128],
                                                ident[:])
                            nc.vector.tensor_copy(MT[b][k][:, i * 128:(i + 1) * 128],
                                                  tp[:])
                for k in range(2):
                    tp = ps2.tile([128, 128], F32, tag="tp", name="tp")
                    nc.tensor.transpose(tp[:], KNc[:, k * 128:(k + 1) * 128], ident[:])
                    nc.vector.tensor_copy(KT[b][k][:], tp[:])
                gps = ps.tile([128, C], F32, tag=f"mm{b}", name=f"gps{b}")
                nc.tensor.matmul(gps[:], KT[b][0][:], KT[b][0][:], start=True, stop=False)
                nc.tensor.matmul(gps[:], KT[b][1][:], KT[b][1][:], start=False, stop=True)
                nc.vector.tensor_tensor(Gsn[b][:], gps[:], maskUneg[:], op=AL.mult)
                aps = ps.tile([C, DV], F32, tag=f"mm{b}", name=f"aps{b}")
                nc.tensor.matmul(aps[:], KT[b][0][:], MT[b][0][:], start=True, stop=False)
                nc.tensor.matmul(aps[:], KT[b][1][:], MT[b][1][:], start=False, stop=True)
                nc.vector.tensor_copy(A[b][:], aps[:])
                nc.vector.memset(colsG[b][:, 0:1], g1c)
                nc.vector.tensor_scalar_mul(colsG[b][:, 1:2], rPm10[:],
                                            -0.1 / (1.0 - D0))
                nc.vector.tensor_copy(colsG[b][:, 2:4], cstPP[:, 0:2])

            if use_mhat:
                mh_ps = ps.tile([128, 8], F32, tag="sm", name="mhps")
                nc.tensor.transpose(mh_ps[:, 0:1], mhat_t[0:1, c0:c0 + C],
                                    ident[0:1, 0:1])
                rmx = wk.tile([128, 1], F32, tag="rmx", name="rmx")
                nc.vector.tensor_scalar_add(rmx[:], mh_ps[:, 0:1], EPS)
                nc.vector.reciprocal(rmx[:], rmx[:])

            for j in range(NSOLVE):
                for b in range(B_LOC):
                    g1 = colsG[b][:, 0:1]
                    q2n = colsG[b][:, 1:2]
                    t1 = etile[b]
                    nc.vector.tensor_scalar_mul(t1[:], A[b][:], g1)
                    nc.vector.scalar_tensor_tensor(
                        R1[b][:], V[b][:, c * DV:(c + 1) * DV], q2n, t1[:],
                        op0=AL.mult, op1=AL.add)
                    for it in range(NIT[j]):
                        if j == 0 and it == 0:
                            nc.vector.tensor_copy(W[b][:], R1[b][:])
                            continue
                        sps = ps.tile([C, DV], F32, tag=f"mm{b}", name=f"sps{b}")
                        nc.tensor.matmul(sps[:], Gsn[b][:], W[b][:],
                                         start=True, stop=True)
                        nc.vector.scalar_tensor_tensor(
                            W[b][:], sps[:], g1, R1[b][:], op0=AL.mult, op1=AL.add)
                if j == NSOLVE - 1:
                    break
                # decay update
                for b in range(B_LOC):
                    Pc = colsG[b][:, 2:3]
                    Vc = V[b][:, c * DV:(c + 1) * DV]
                    nc.vector.tensor_scalar_mul(utile[b][:], W[b][:], Pc)
                    nc.vector.tensor_tensor(etile[b][:], utile[b][:], Vc,
                                            op=AL.subtract)
                    nc.scalar.activation(sjunk[:], etile[b][:], AF.Square,
                                         accum_out=colsG[b][:, 6:7],
                                         scale=1.0 / 1.1)
                    nc.scalar.activation(sjunk[:], utile[b][:], AF.Square,
                                         accum_out=colsG[b][:, 5:6])
                    nc.scalar.sqrt(colsG[b][:, 4:5], colsG[b][:, 6:7])
                if use_mhat:
                    rmxc = rmx
                else:
                    mxc = wk.tile([128, 1], F32, tag="mxc", name="mxc")
                    nc.vector.tensor_tensor(mxc[:], colsG[0][:, 4:5],
                                            colsG[1][:, 4:5], op=AL.max)
                    nc.vector.tensor_scalar_add(mxc[:], mxc[:], EPS)
                    rmxc = wk.tile([128, 1], F32, tag="rmxc", name="rmxc")
                    nc.vector.reciprocal(rmxc[:], mxc[:])
                for b in range(B_LOC):
                    u2 = colsG[b][:, 5:6]
                    s2 = colsG[b][:, 6:7]
                    sc = colsG[b][:, 7:8]
                    nc.vector.tensor_scalar(sc, s2, -0.605, None, op0=AL.mult)
                    nc.vector.scalar_tensor_tensor(sc, v2[b][:, c:c + 1], 0.5, sc,
                                                   op0=AL.mult, op1=AL.add)
                    nc.vector.scalar_tensor_tensor(sc, u2, 0.5, sc,
                                                   op0=AL.mult, op1=AL.add)
                    t5 = wk.tile([128, 1], F32, tag=f"t5{b}", name=f"t5{b}")
                    nc.vector.tensor_scalar_mul(t5[:], u2, 1.0 / 1.1)
                    nc.vector.scalar_tensor_tensor(sc, sc, 0.1 / 1.1, t5[:],
                                                   op0=AL.mult, op1=AL.add)
                    omd = wk.tile([128, 1], F32, tag=f"omd{b}", name=f"omd{b}")
                    nc.vector.reciprocal(omd[:], colsG[b][:, 0:1])
                    nc.vector.tensor_tensor(sc, sc, omd[:], op=AL.mult)
                    t6 = wk.tile([128, 1], F32, tag=f"t6{b}", name=f"t6{b}")
                    nc.vector.tensor_tensor(t6[:], u2, knsq[b][:, c:c + 1],
                                            op=AL.mult)
                    nc.vector.scalar_tensor_tensor(COLP[:, 2 + b:3 + b], sc, -2.2,
                                                   t6[:], op0=AL.mult, op1=AL.add)
                    nc.vector.tensor_tensor(t5[:], omd[:], omd[:], op=AL.mult)
                    nc.vector.tensor_scalar_mul(COLP[:, 0 + b:1 + b], t5[:], 1.21)
                    nc.vector.tensor_tensor(COLP[:, 4 + b:5 + b], colsG[b][:, 4:5],
                                            rmxc[:], op=AL.mult)
                    if not use_mhat and j == NSOLVE - 2:
                        nc.vector.tensor_copy(snall[b][:, c:c + 1], colsG[b][:, 4:5])
                tps = ps2.tile([128, 3 * C], F32, tag="tp", name="tps")
                for q in range(3):
                    nc.tensor.transpose(tps[0:2, q * C:(q + 1) * C],
                                        COLP[:, 2 * q:2 * q + 2], ident[:])
                nc.vector.tensor_copy(ROWP[0:2, :], tps[0:2, 0:3 * C])
                n2cur = N2tiles[(c % 2) * 2 + j]
                nc.vector.tensor_tensor_scan(n2cur[:], ROWP[:, 0:C], ROWP[:, C:2 * C],
                                             carry_ap, op0=AL.mult, op1=AL.add)
                utr = wk.tile([2, 2 * C], F32, tag="utr", name="utr")
                nc.vector.tensor_scalar_max(utr[:, 0:C], n2cur[:], 0.0)
                nc.scalar.activation(utr[:, C:2 * C], utr[:, 0:C], AF.Sqrt,
                                     scale=1.0 / (MAXN_EPS * MAXN_EPS))
                nc.vector.tensor_scalar_min(utr[:, 0:C], utr[:, C:2 * C], 1.0)
                drow = wk.tile([2, C], F32, tag="drow", name="drow")
                nc.vector.tensor_scalar(drow[:, :], utr[:, 0:C], 0.001, 0.01,
                                        op0=AL.mult, op1=AL.add)
                nc.vector.scalar_tensor_tensor(drow[:, :], ROWP[:, 2 * C:3 * C], 0.001,
                                               drow[:, :], op0=AL.mult, op1=AL.add)
                nc.vector.tensor_scalar(ROWP2[:, 0:C], drow[:, :], -1.0, 1.0,
                                        op0=AL.mult, op1=AL.add)
                nc.vector.tensor_tensor_scan(ROWP2[:, C:2 * C], ROWP2[:, 0:C],
                                             zeros2[0:2, :], 1.0,
                                             op0=AL.mult, op1=AL.add)
                nc.vector.memset(ROWP2[:, 2 * C:2 * C + 1], 1.0)
                nc.vector.tensor_copy(ROWP2[:, 2 * C + 1:3 * C], ROWP2[:, C:2 * C - 1])
                tps2 = ps.tile([128, 8], F32, tag="sm", name="tps2")
                for q in range(3):
                    nc.tensor.transpose(tps2[:, 2 * q:2 * q + 2],
                                        ROWP2[0:2, q * C:(q + 1) * C], ident[0:2, 0:2])
                nc.vector.tensor_copy(COL2[:, 0:6], tps2[:, 0:6])
                for b in range(B_LOC):
                    omdc = COL2[:, 0 + b:1 + b]
                    nc.vector.reciprocal(colsG[b][:, 7:8], omdc)
                    nc.vector.tensor_scalar_mul(colsG[b][:, 0:1], colsG[b][:, 7:8],
                                                1.1)
                    nc.vector.tensor_copy(colsG[b][:, 2:3], COL2[:, 2 + b:3 + b])
                    nc.vector.tensor_copy(colsG[b][:, 3:4], COL2[:, 4 + b:5 + b])
                    rpm = wk.tile([128, 1], F32, tag=f"rpm{b}", name=f"rpm{b}")
                    nc.vector.reciprocal(rpm[:], COL2[:, 4 + b:5 + b])
                    nc.vector.tensor_tensor(rpm[:], rpm[:], colsG[b][:, 7:8],
                                            op=AL.mult)
                    nc.vector.tensor_scalar_mul(colsG[b][:, 1:2], rpm[:], -0.1)
                if j == NSOLVE - 2:
                    carry_next = n2cur[:, C - 1:C]
            carry_ap = carry_next

            # state update
            for b in range(B_LOC):
                bps = ps.tile([128, 8], F32, tag="sm", name="bps")
                nc.tensor.matmul(bps[:, 0:1], sel127[:], colsG[b][:, 2:3],
                                 start=True, stop=True)
                PCc = wk.tile([128, 1], F32, tag=f"pcc{b}", name=f"pcc{b}")
                nc.vector.tensor_copy(PCc[:], bps[:, 0:1])
                Wn = etile[b]
                nc.vector.tensor_scalar_mul(Wn[:], W[b][:], -1.0)
                KNc = KnN[b][:, c * DK:(c + 1) * DK]
                for i in range(2):
                    mps = ps.tile([128, DV], F32, tag=f"mm{b}", name=f"mps{b}")
                    nc.tensor.matmul(mps[:], KNc[:, i * 128:(i + 1) * 128], Wn[:],
                                     start=True, stop=False)
                    nc.tensor.matmul(mps[:], ident[:], MT[b][i][:],
                                     start=False, stop=True)
                    nc.vector.tensor_scalar_mul(MT[b][i][:], mps[:], PCc[:])

        for b in range(B_LOC):
            if not use_mhat:
                nc.sync.dma_start(nrm_d[b, :, :], snall[b][:])
            for i in range(2):
                st = per.tile([128, DK], F32, tag=f"st{b}{i}", name=f"st{b}{i}")
                for k in range(2):
                    tp = ps2.tile([128, 128], F32, tag="tp", name="tp")
                    nc.tensor.transpose(tp[:], MT[b][k][:, i * 128:(i + 1) * 128],
                                        ident[:])
                    nc.vector.tensor_copy(st[:, k * 128:(k + 1) * 128], tp[:])
                nc.sync.dma_start(out_d[b, i * 128:(i + 1) * 128, :], st[:])
    return nc


def _build(use_mhat):
    key = ("nc", use_mhat)
    if key not in _cache:
        nc = bacc.Bacc("TRN2", target_bir_lowering=False, debug=False, num_devices=8)
        _emit(nc, use_mhat)
        nc.compile()
        _cache[key] = nc
    return _cache[key]


def kernel(memory, keys, values):
    memory = np.ascontiguousarray(memory, np.float32)
    keys = np.ascontiguousarray(keys, np.float32)
    values = np.ascontiguousarray(values, np.float32)
    B = memory.shape[0]
    n2 = (memory.astype(np.float64) ** 2).sum(axis=(1, 2)).astype(np.float32)

    def in_maps(mhat):
        maps = []
        for ci in range(8):
            sl = slice(ci * B_LOC, (ci + 1) * B_LOC)
            maps.append({
                "keys": np.ascontiguousarray(keys[sl]),
                "vals": np.ascontiguousarray(values[sl]),
                "mem": np.ascontiguousarray(memory[sl]),
                "n2in": np.ascontiguousarray(n2[sl].reshape(B_LOC, 1)),
                "mhat": mhat,
            })
        return maps

    zero_mhat = np.zeros((1, S), np.float32)
    nc1 = _build(False)
    r1 = run_bass_kernel_spmd(nc1, in_maps(zero_mhat), core_ids=list(range(8)))
    allnorms = np.concatenate([r["nrm"] for r in r1.results], axis=0)
    norms_t = allnorms.transpose(0, 2, 1).reshape(B, S)
    mhat = np.ascontiguousarray(norms_t.max(axis=0).reshape(1, S).astype(np.float32))

    nc2 = _build(True)
    r2 = run_bass_kernel_spmd(nc2, in_maps(mhat), core_ids=list(range(8)))
    out = np.concatenate([r["out"] for r in r2.results], axis=0)
    return out


# revision 8
# speedup vs baseline: 1.2189x; 1.2189x over previous
"""DynamicDecayMemory Trainium2 kernel.

Full inputs: memory (16,256,256), keys (16,4096,256), values (16,4096,256).
Data-parallel over batch: 8 cores x 2 batches each. The sequential scan is
reformulated as chunked (C=128) triangular solves in "w-space"
(u_t = P_t * w_t, P = cumprod(1-d)) solved by Neumann iteration with the
kn-Gram matrix; decay d_t recovered via a small fixed point. The global
cross-batch max of surprise norms is handled with two launches: pass 1 uses
the local 2-batch max and outputs per-step surprise norms; the host reduces
the global per-step max; pass 2 consumes it (validated ~1e-5 rel err).
"""
import sys
import numpy as np

sys.path.insert(0, "/opt/trn_rl_repo")

import concourse.bass as bass
import concourse.bacc as bacc
import concourse.mybir as mybir
import concourse.tile as tile
from concourse import masks
from concourse.bass_utils import run_bass_kernel_spmd
from contextlib import ExitStack

F32 = mybir.dt.float32
F32R = mybir.dt.float32r
AL = mybir.AluOpType
AF = mybir.ActivationFunctionType

B_LOC = 2
S = 4096
C = 128
NCH = S // C
DK = 256
DV = 256
EPS = 1e-6
MAXN_EPS = 256.0 + EPS
D0 = 0.0108

_cache = {}


def _emit(nc, use_mhat):
    keys_d = nc.dram_tensor("keys", [B_LOC, S, DK], F32, kind="ExternalInput")
    vals_d = nc.dram_tensor("vals", [B_LOC, S, DV], F32, kind="ExternalInput")
    mem_d = nc.dram_tensor("mem", [B_LOC, DV, DK], F32, kind="ExternalInput")
    n2in_d = nc.dram_tensor("n2in", [B_LOC, 1], F32, kind="ExternalInput")
    mhat_d = nc.dram_tensor("mhat", [1, S], F32, kind="ExternalInput")
    out_d = nc.dram_tensor("out", [B_LOC, DV, DK], F32, kind="ExternalOutput")
    nrm_d = nc.dram_tensor("nrm", [B_LOC, C, NCH], F32, kind="ExternalOutput")

    NSOLVE = 3 if use_mhat else 2
    NIT = [5, 4, 4]

    with tile.TileContext(nc) as tc, ExitStack() as ctx:
        per = ctx.enter_context(tc.tile_pool(name="per", bufs=1))
        wk = ctx.enter_context(tc.tile_pool(name="wk", bufs=2))
        ps = ctx.enter_context(tc.tile_pool(name="ps", bufs=1, space="PSUM"))
        ps2 = ctx.enter_context(tc.tile_pool(name="ps2", bufs=2, space="PSUM"))

        KnN = [per.tile([C, NCH * DK], F32, tag=f"kn{b}", name=f"kn{b}") for b in range(B_LOC)]
        V = [per.tile([C, NCH * DV], F32, tag=f"v{b}", name=f"v{b}") for b in range(B_LOC)]
        MT = [[per.tile([128, DV], F32, tag=f"mt{b}{i}", name=f"mt{b}{i}") for i in range(2)]
              for b in range(B_LOC)]
        knsq = [per.tile([C, NCH], F32, tag=f"ksq{b}", name=f"ksq{b}") for b in range(B_LOC)]
        v2 = [per.tile([C, NCH], F32, tag=f"v2{b}", name=f"v2{b}") for b in range(B_LOC)]
        snall = [per.tile([C, NCH], F32, tag=f"sna{b}", name=f"sna{b}") for b in range(B_LOC)]

        ident = per.tile([128, 128], F32, tag="ident", name="ident")
        masks.make_identity(nc, ident[:])
        maskUneg = per.tile([128, 128], F32, tag="msku", name="msku")
        masks.make_upper_triangular(nc, maskUneg[:], val=-1.0, diag=False)
        sel127 = per.tile([128, 128], F32, tag="sel127", name="sel127")
        nc.gpsimd.memset(sel127[:], 0.0)
        nc.gpsimd.affine_select(out=sel127[:], in_=sel127[:],
                                compare_op=AL.not_equal, fill=1.0, base=-127,
                                pattern=[[0, 128]], channel_multiplier=1)
        # absorber: a PE op depending only on Pool-made tiles, so later fp32
        # matmuls need just one (DVE) wait.
        absps = ps2.tile([128, 128], F32, tag="tp", name="absps")
        nc.tensor.transpose(absps[:], ident[:], ident[:])

        zeros2 = per.tile([8, C], F32, tag="zr", name="zr")
        nc.vector.memset(zeros2[:], 0.0)
        mhat_t = per.tile([1, S], F32, tag="mhat", name="mhat")
        if use_mhat:
            nc.sync.dma_start(mhat_t[:], mhat_d[:])
        n2in_t = per.tile([B_LOC, 1], F32, tag="n2in", name="n2in")
        nc.sync.dma_start(n2in_t[:], n2in_d[:])

        # constant-d columns: P0/Pm10 as [128,1] cols via row-scan + transpose
        d0row = per.tile([2, 3 * C], F32, tag="d0r", name="d0r")
        nc.vector.memset(d0row[:, 0:C], 1.0 - D0)
        nc.vector.tensor_tensor_scan(d0row[:, C:2 * C], d0row[:, 0:C],
                                     zeros2[0:2, :], 1.0, op0=AL.mult, op1=AL.add)
        nc.vector.memset(d0row[:, 2 * C:2 * C + 1], 1.0)
        nc.vector.tensor_copy(d0row[:, 2 * C + 1:3 * C], d0row[:, C:2 * C - 1])
        pk_ps = ps.tile([128, 8], F32, tag="sm", name="pk")
        nc.tensor.transpose(pk_ps[:, 0:2], d0row[0:2, C:2 * C], ident[0:2, 0:2])
        nc.tensor.transpose(pk_ps[:, 2:4], d0row[0:2, 2 * C:3 * C], ident[0:2, 0:2])
        cstPP = per.tile([128, 2], F32, tag="cstpp", name="cstpp")
        nc.vector.tensor_copy(cstPP[:, 0:1], pk_ps[:, 0:1])   # P0 col
        nc.vector.tensor_copy(cstPP[:, 1:2], pk_ps[:, 2:3])   # Pm10 col
        rPm10 = per.tile([128, 1], F32, tag="rpm0", name="rpm0")
        nc.vector.reciprocal(rPm10[:], cstPP[:, 1:2])
        g1c = 1.1 / (1.0 - D0)

        N2tiles = [per.tile([2, C], F32, tag=f"n2_{i}", name=f"n2_{i}") for i in range(4)]
        carry_ap = n2in_t[:]

        for c in range(NCH):
            c0 = c * C
            KT = [[wk.tile([128, C], F32, tag=f"kt{b}{i}", name=f"kt{b}{i}") for i in range(2)]
                  for b in range(B_LOC)]
            Gsn = [wk.tile([128, C], F32R, tag=f"g{b}", name=f"g{b}") for b in range(B_LOC)]
            A = [wk.tile([C, DV], F32, tag=f"a{b}", name=f"a{b}") for b in range(B_LOC)]
            W = [wk.tile([C, DV], F32R, tag=f"w{b}", name=f"w{b}") for b in range(B_LOC)]
            R1 = [wk.tile([C, DV], F32, tag=f"r1{b}", name=f"r1{b}") for b in range(B_LOC)]
            etile = [wk.tile([C, DV], F32, tag=f"e{b}", name=f"e{b}") for b in range(B_LOC)]
            utile = [wk.tile([C, DV], F32, tag=f"u{b}", name=f"u{b}") for b in range(B_LOC)]
            sjunk = wk.tile([C, DV], F32, tag="sj", name="sj")
            colsG = [wk.tile([128, 8], F32, tag=f"cg{b}", name=f"cg{b}") for b in range(B_LOC)]
            COLP = wk.tile([128, 6], F32, tag="colp", name="colp")
            ROWP = wk.tile([2, 3 * C], F32, tag="rowp", name="rowp")
            ROWP2 = wk.tile([2, 3 * C], F32, tag="rowp2", name="rowp2")
            COL2 = wk.tile([128, 6], F32, tag="col2", name="col2")

            for b in range(B_LOC):
                KNc = KnN[b][:, c * DK:(c + 1) * DK]
                Vc = V[b][:, c * DV:(c + 1) * DV]
                ktmp = wk.tile([C, DK], F32, tag=f"ktmp{b}", name=f"ktmp{b}")
                nc.sync.dma_start(ktmp[:], keys_d[b, c0:c0 + C, :])
                nc.sync.dma_start(Vc, vals_d[b, c0:c0 + C, :])
                nrm2 = wk.tile([C, 1], F32, tag=f"nn{b}", name=f"nn{b}")
                nc.scalar.activation(sjunk[:], ktmp[:], AF.Square, accum_out=nrm2[:])
                nrm = wk.tile([C, 1], F32, tag=f"nr{b}", name=f"nr{b}")
                nc.scalar.sqrt(nrm[:], nrm2[:])
                nrme = wk.tile([C, 1], F32, tag=f"ne{b}", name=f"ne{b}")
                nc.vector.tensor_scalar_add(nrme[:], nrm[:], EPS)
                rk = wk.tile([C, 1], F32, tag=f"rk{b}", name=f"rk{b}")
                nc.vector.reciprocal(rk[:], nrme[:])
                nc.vector.tensor_scalar_mul(KNc, ktmp[:], rk[:])
                t0 = wk.tile([C, 1], F32, tag=f"t0{b}", name=f"t0{b}")
                nc.vector.tensor_tensor(t0[:], nrm[:], rk[:], op=AL.mult)
                nc.vector.tensor_tensor(knsq[b][:, c:c + 1], t0[:], t0[:], op=AL.mult)
                nc.scalar.activation(sjunk[:], Vc, AF.Square,
                                     accum_out=v2[b][:, c:c + 1])
                if c == 0:
                    for i in range(2):
                        mnat = wk.tile([128, DK], F32, tag=f"mn{b}", name=f"mn{b}")
                        nc.sync.dma_start(mnat[:], mem_d[b, i * 128:(i + 1) * 128, :])
                        for k in range(2):
                            tp = ps2.tile([128, 128], F32, tag="tp", name="tp")
                            nc.tensor.transpose(tp[:], mnat[:, k * 128:(k + 1) * 128],
                                                ident[:])
                            nc.vector.tensor_copy(MT[b][k][:, i * 128:(i + 1) * 128],
                                                  tp[:])
                for k in range(2):
                    tp = ps2.tile([128, 128], F32, tag="tp", name="tp")
                    nc.tensor.transpose(tp[:], KNc[:, k * 128:(k + 1) * 128], ident[:])
                    nc.vector.tensor_copy(KT[b][k][:], tp[:])
                gps = ps.tile([128, C], F32, tag=f"mm{b}", name=f"gps{b}")
                nc.tensor.matmul(gps[:], KT[b][0][:], KT[b][0][:], start=True, stop=False)
                nc.tensor.matmul(gps[:], KT[b][1][:], KT[b][1][:], start=False, stop=True)
                nc.vector.tensor_tensor(Gsn[b][:], gps[:], maskUneg[:], op=AL.mult)
                aps = ps.tile([C, DV], F32, tag=f"mm{b}", name=f"aps{b}")
                nc.tensor.matmul(aps[:], KT[b][0][:], MT[b][0][:], start=True, stop=False)
                nc.tensor.matmul(aps[:], KT[b][1][:], MT[b][1][:], start=False, stop=True)
                nc.vector.tensor_copy(A[b][:], aps[:])
                nc.vector.memset(colsG[b][:, 0:1], g1c)
                nc.vector.tensor_scalar_mul(colsG[b][:, 1:2], rPm10[:],
                                            -0.1 / (1.0 - D0))
                nc.vector.tensor_copy(colsG[b][:, 2:4], cstPP[:, 0:2])

            if use_mhat:
                mh_ps = ps.tile([128, 8], F32, tag="sm", name="mhps")
                nc.tensor.transpose(mh_ps[:, 0:1], mhat_t[0:1, c0:c0 + C],
                                    ident[0:1, 0:1])
                rmx = wk.tile([128, 1], F32, tag="rmx", name="rmx")
                nc.vector.tensor_scalar_add(rmx[:], mh_ps[:, 0:1], EPS)
                nc.vector.reciprocal(rmx[:], rmx[:])

            for j in range(NSOLVE):
                for b in range(B_LOC):
                    g1 = colsG[b][:, 0:1]
                    q2n = colsG[b][:, 1:2]
                    t1 = etile[b]
                    nc.vector.tensor_scalar_mul(t1[:], A[b][:], g1)
                    nc.vector.scalar_tensor_tensor(
                        R1[b][:], V[b][:, c * DV:(c + 1) * DV], q2n, t1[:],
                        op0=AL.mult, op1=AL.add)
                    for it in range(NIT[j]):
                        if j == 0 and it == 0:
                            nc.vector.tensor_copy(W[b][:], R1[b][:])
                            continue
                        sps = ps.tile([C, DV], F32, tag=f"mm{b}", name=f"sps{b}")
                        nc.tensor.matmul(sps[:], Gsn[b][:], W[b][:],
                                         start=True, stop=True)
                        nc.vector.scalar_tensor_tensor(
                            W[b][:], sps[:], g1, R1[b][:], op0=AL.mult, op1=AL.add)
                if j == NSOLVE - 1:
                    break
                # decay update
                for b in range(B_LOC):
                    Pc = colsG[b][:, 2:3]
                    Vc = V[b][:, c * DV:(c + 1) * DV]
                    nc.vector.tensor_scalar_mul(utile[b][:], W[b][:], Pc)
                    nc.vector.tensor_tensor(etile[b][:], utile[b][:], Vc,
                                            op=AL.subtract)
                    nc.scalar.activation(sjunk[:], etile[b][:], AF.Square,
                                         accum_out=colsG[b][:, 6:7],
                                         scale=1.0 / 1.1)
                    nc.scalar.activation(sjunk[:], utile[b][:], AF.Square,
                                         accum_out=colsG[b][:, 5:6])
                    nc.scalar.sqrt(colsG[b][:, 4:5], colsG[b][:, 6:7])
                if use_mhat:
                    rmxc = rmx
                else:
                    mxc = wk.tile([128, 1], F32, tag="mxc", name="mxc")
                    nc.vector.tensor_tensor(mxc[:], colsG[0][:, 4:5],
                                            colsG[1][:, 4:5], op=AL.max)
                    nc.vector.tensor_scalar_add(mxc[:], mxc[:], EPS)
                    rmxc = wk.tile([128, 1], F32, tag="rmxc", name="rmxc")
                    nc.vector.reciprocal(rmxc[:], mxc[:])
                for b in range(B_LOC):
                    u2 = colsG[b][:, 5:6]
                    s2 = colsG[b][:, 6:7]
                    sc = colsG[b][:, 7:8]
                    nc.vector.tensor_scalar(sc, s2, -0.605, None, op0=AL.mult)
                    nc.vector.scalar_tensor_tensor(sc, v2[b][:, c:c + 1], 0.5, sc,
                                                   op0=AL.mult, op1=AL.add)
                    nc.vector.scalar_tensor_tensor(sc, u2, 0.5, sc,
                                                   op0=AL.mult, op1=AL.add)
                    t5 = wk.tile([128, 1], F32, tag=f"t5{b}", name=f"t5{b}")
                    nc.vector.tensor_scalar_mul(t5[:], u2, 1.0 / 1.1)
                    nc.vector.scalar_tensor_tensor(sc, sc, 0.1 / 1.1, t5[:],
                                                   op0=AL.mult, op1=AL.add)
                    omd = wk.tile([128, 1], F32, tag=f"omd{b}", name=f"omd{b}")
                    nc.vector.reciprocal(omd[:], colsG[b][:, 0:1])
                    nc.vector.tensor_tensor(sc, sc, omd[:], op=AL.mult)
                    t6 = wk.tile([128, 1], F32, tag=f"t6{b}", name=f"t6{b}")
                    nc.vector.tensor_tensor(t6[:], u2, knsq[b][:, c:c + 1],
                                            op=AL.mult)
                    nc.vector.scalar_tensor_tensor(COLP[:, 2 + b:3 + b], sc, -2.2,
                                                   t6[:], op0=AL.mult, op1=AL.add)
                    nc.vector.tensor_tensor(t5[:], omd[:], omd[:], op=AL.mult)
                    nc.vector.tensor_scalar_mul(COLP[:, 0 + b:1 + b], t5[:], 1.21)
                    nc.vector.tensor_tensor(COLP[:, 4 + b:5 + b], colsG[b][:, 4:5],
                                            rmxc[:], op=AL.mult)
                    if not use_mhat and j == NSOLVE - 2:
                        nc.vector.tensor_copy(snall[b][:, c:c + 1], colsG[b][:, 4:5])
                tps = ps2.tile([128, 3 * C], F32, tag="tp", name="tps")
                for q in range(3):
                    nc.tensor.transpose(tps[0:2, q * C:(q + 1) * C],
                                        COLP[:, 2 * q:2 * q + 2], ident[:])
                nc.vector.tensor_copy(ROWP[0:2, :], tps[0:2, 0:3 * C])
                n2cur = N2tiles[(c % 2) * 2 + j]
                nc.vector.tensor_tensor_scan(n2cur[:], ROWP[:, 0:C], ROWP[:, C:2 * C],
                                             carry_ap, op0=AL.mult, op1=AL.add)
                utr = wk.tile([2, 2 * C], F32, tag="utr", name="utr")
                nc.vector.tensor_scalar_max(utr[:, 0:C], n2cur[:], 0.0)
                nc.scalar.activation(utr[:, C:2 * C], utr[:, 0:C], AF.Sqrt,
                                     scale=1.0 / (MAXN_EPS * MAXN_EPS))
                nc.vector.tensor_scalar_min(utr[:, 0:C], utr[:, C:2 * C], 1.0)
                drow = wk.tile([2, C], F32, tag="drow", name="drow")
                nc.vector.tensor_scalar(drow[:, :], utr[:, 0:C], 0.001, 0.01,
                                        op0=AL.mult, op1=AL.add)
                nc.vector.scalar_tensor_tensor(drow[:, :], ROWP[:, 2 * C:3 * C], 0.001,
                                               drow[:, :], op0=AL.mult, op1=AL.add)
                nc.vector.tensor_scalar(ROWP2[:, 0:C], drow[:, :], -1.0, 1.0,
                                        op0=AL.mult, op1=AL.add)
                nc.vector.tensor_tensor_scan(ROWP2[:, C:2 * C], ROWP2[:, 0:C],
                                             zeros2[0:2, :], 1.0,
                                             op0=AL.mult, op1=AL.add)
                nc.vector.memset(ROWP2[:, 2 * C:2 * C + 1], 1.0)
                nc.vector.tensor_copy(ROWP2[:, 2 * C + 1:3 * C], ROWP2[:, C:2 * C - 1])
                tps2 = ps.tile([128, 8], F32, tag="sm", name="tps2")
                for q in range(3):
                    nc.tensor.transpose(tps2[:, 2 * q:2 * q + 2],
                                        ROWP2[0:2, q * C:(q + 1) * C], ident[0:2, 0:2])
                nc.vector.tensor_copy(COL2[:, 0:6], tps2[:, 0:6])
                for b in range(B_LOC):
                    omdc = COL2[:, 0 + b:1 + b]
                    nc.vector.reciprocal(colsG[b][:, 7:8], omdc)
                    nc.vector.tensor_scalar_mul(colsG[b][:, 0:1], colsG[b][:, 7:8],
                                                1.1)
                    nc.vector.tensor_copy(colsG[b][:, 2:3], COL2[:, 2 + b:3 + b])
                    nc.vector.tensor_copy(colsG[b][:, 3:4], COL2[:, 4 + b:5 + b])
                    rpm = wk.tile([128, 1], F32, tag=f"rpm{b}", name=f"rpm{b}")
                    nc.vector.reciprocal(rpm[:], COL2[:, 4 + b:5 + b])
                    nc.vector.tensor_tensor(rpm[:], rpm[:], colsG[b][:, 7:8],
                                            op=AL.mult)
                    nc.vector.tensor_scalar_mul(colsG[b][:, 1:2], rpm[:], -0.1)
                if j == NSOLVE - 2:
                    carry_next = n2cur[:, C - 1:C]
            carry_ap = carry_next

            # state update
            for b in range(B_LOC):
                bps = ps.tile([128, 8], F32, tag="sm", name="bps")
                nc.tensor.matmul(bps[:, 0:1], sel127[:], colsG[b][:, 2:3],
                                 start=True, stop=True)
                PCc = wk.tile([128, 1], F32, tag=f"pcc{b}", name=f"pcc{b}")
                nc.vector.tensor_copy(PCc[:], bps[:, 0:1])
                Wn = etile[b]
                nc.vector.tensor_scalar_mul(Wn[:], W[b][:], -1.0)
                KNc = KnN[b][:, c * DK:(c + 1) * DK]
                for i in range(2):
                    mps = ps.tile([128, DV], F32, tag=f"mm{b}", name=f"mps{b}")
                    nc.tensor.matmul(mps[:], KNc[:, i * 128:(i + 1) * 128], Wn[:],
                                     start=True, stop=False)
                    nc.tensor.matmul(mps[:], ident[:], MT[b][i][:],
                                     start=False, stop=True)
                    nc.vector.tensor_scalar_mul(MT[b][i][:], mps[:], PCc[:])

        for b in range(B_LOC):
            if not use_mhat:
                nc.sync.dma_start(nrm_d[b, :, :], snall[b][:])
            for i in range(2):
                st = per.tile([128, DK], F32, tag=f"st{b}{i}", name=f"st{b}{i}")
                for k in range(2):
                    tp = ps2.tile([128, 128], F32, tag="tp", name="tp")
                    nc.tensor.transpose(tp[:], MT[b][k][:, i * 128:(i + 1) * 128],
                                        ident[:])
                    nc.vector.tensor_copy(st[:, k * 128:(k + 1) * 128], tp[:])
                nc.sync.dma_start(out_d[b, i * 128:(i + 1) * 128, :], st[:])
    return nc


def _build(use_mhat):
    key = ("nc", use_mhat)
    if key not in _cache:
        nc = bacc.Bacc("TRN2", target_bir_lowering=False, debug=False, num_devices=8)
        _emit(nc, use_mhat)
        nc.compile()
        _cache[key] = nc
    return _cache[key]


def kernel(memory, keys, values):
    memory = np.ascontiguousarray(memory, np.float32)
    keys = np.ascontiguousarray(keys, np.float32)
    values = np.ascontiguousarray(values, np.float32)
    B = memory.shape[0]
    n2 = (memory.astype(np.float64) ** 2).sum(axis=(1, 2)).astype(np.float32)

    def in_maps(mhat):
        maps = []
        for ci in range(8):
            sl = slice(ci * B_LOC, (ci + 1) * B_LOC)
            maps.append({
                "keys": np.ascontiguousarray(keys[sl]),
                "vals": np.ascontiguousarray(values[sl]),
                "mem": np.ascontiguousarray(memory[sl]),
                "n2in": np.ascontiguousarray(n2[sl].reshape(B_LOC, 1)),
                "mhat": mhat,
            })
        return maps

    zero_mhat = np.zeros((1, S), np.float32)
    nc1 = _build(False)
    r1 = run_bass_kernel_spmd(nc1, in_maps(zero_mhat), core_ids=list(range(8)))
    allnorms = np.concatenate([r["nrm"] for r in r1.results], axis=0)
    norms_t = allnorms.transpose(0, 2, 1).reshape(B, S)
    mhat = np.ascontiguousarray(norms_t.max(axis=0).reshape(1, S).astype(np.float32))

    nc2 = _build(True)
    r2 = run_bass_kernel_spmd(nc2, in_maps(mhat), core_ids=list(range(8)))
    out = np.concatenate([r["out"] for r in r2.results], axis=0)
    return out


# revision 9
# speedup vs baseline: 1.3829x; 1.1345x over previous
"""DynamicDecayMemory Trainium2 kernel.

Full inputs: memory (16,256,256), keys (16,4096,256), values (16,4096,256).
Data-parallel over batch: 8 cores x 2 batches each. The sequential scan is
reformulated as chunked (C=128) triangular solves in "w-space"
(u_t = P_t * w_t, P = cumprod(1-d)) solved by Neumann iteration with the
kn-Gram matrix; decay d_t recovered via a small fixed point. The global
cross-batch max of surprise norms is handled with two launches: pass 1 uses
the local 2-batch max and outputs per-step surprise norms; the host reduces
the global per-step max; pass 2 consumes it (validated ~1e-5 rel err).
"""
import sys
import numpy as np

sys.path.insert(0, "/opt/trn_rl_repo")

import concourse.bass as bass
import concourse.bacc as bacc
import concourse.mybir as mybir
import concourse.tile as tile
from concourse import masks
from concourse.bass_utils import run_bass_kernel_spmd
from contextlib import ExitStack

F32 = mybir.dt.float32
F32R = mybir.dt.float32r
AL = mybir.AluOpType
AF = mybir.ActivationFunctionType

B_LOC = 2
S = 4096
C = 128
NCH = S // C
DK = 256
DV = 256
EPS = 1e-6
MAXN_EPS = 256.0 + EPS
D0 = 0.0108

_cache = {}


def _emit(nc, use_mhat):
    keys_d = nc.dram_tensor("keys", [B_LOC, S, DK], F32, kind="ExternalInput")
    vals_d = nc.dram_tensor("vals", [B_LOC, S, DV], F32, kind="ExternalInput")
    mem_d = nc.dram_tensor("mem", [B_LOC, DV, DK], F32, kind="ExternalInput")
    n2in_d = nc.dram_tensor("n2in", [B_LOC, 1], F32, kind="ExternalInput")
    mhat_d = nc.dram_tensor("mhat", [1, S], F32, kind="ExternalInput")
    out_d = nc.dram_tensor("out", [B_LOC, DV, DK], F32, kind="ExternalOutput")
    nrm_d = nc.dram_tensor("nrm", [B_LOC, C, NCH], F32, kind="ExternalOutput")

    NSOLVE = 3 if use_mhat else 2
    NIT = [5, 4, 4]

    with tile.TileContext(nc) as tc, ExitStack() as ctx:
        per = ctx.enter_context(tc.tile_pool(name="per", bufs=1))
        wk = ctx.enter_context(tc.tile_pool(name="wk", bufs=2))
        ps = ctx.enter_context(tc.tile_pool(name="ps", bufs=1, space="PSUM"))
        ps2 = ctx.enter_context(tc.tile_pool(name="ps2", bufs=2, space="PSUM"))

        KnN = [per.tile([C, NCH * DK], F32, tag=f"kn{b}", name=f"kn{b}") for b in range(B_LOC)]
        V = [per.tile([C, NCH * DV], F32, tag=f"v{b}", name=f"v{b}") for b in range(B_LOC)]
        MT = [[per.tile([128, DV], F32, tag=f"mt{b}{i}", name=f"mt{b}{i}") for i in range(2)]
              for b in range(B_LOC)]
        knsq = [per.tile([C, NCH], F32, tag=f"ksq{b}", name=f"ksq{b}") for b in range(B_LOC)]
        v2 = [per.tile([C, NCH], F32, tag=f"v2{b}", name=f"v2{b}") for b in range(B_LOC)]
        snall = [per.tile([C, NCH], F32, tag=f"sna{b}", name=f"sna{b}") for b in range(B_LOC)]

        ident = per.tile([128, 128], F32, tag="ident", name="ident")
        masks.make_identity(nc, ident[:])
        maskUneg = per.tile([128, 128], F32, tag="msku", name="msku")
        masks.make_upper_triangular(nc, maskUneg[:], val=-1.0, diag=False)
        sel127 = per.tile([128, 128], F32, tag="sel127", name="sel127")
        nc.gpsimd.memset(sel127[:], 0.0)
        nc.gpsimd.affine_select(out=sel127[:], in_=sel127[:],
                                compare_op=AL.not_equal, fill=1.0, base=-127,
                                pattern=[[0, 128]], channel_multiplier=1)
        # absorber: a PE op depending only on Pool-made tiles, so later fp32
        # matmuls need just one (DVE) wait.
        absps = ps2.tile([128, 128], F32, tag="tp", name="absps")
        nc.tensor.transpose(absps[:], ident[:], ident[:])

        zeros2 = per.tile([8, C], F32, tag="zr", name="zr")
        nc.vector.memset(zeros2[:], 0.0)
        mhat_t = per.tile([1, S], F32, tag="mhat", name="mhat")
        if use_mhat:
            nc.sync.dma_start(mhat_t[:], mhat_d[:])
        n2in_t = per.tile([B_LOC, 1], F32, tag="n2in", name="n2in")
        nc.sync.dma_start(n2in_t[:], n2in_d[:])

        # constant-d columns: P0/Pm10 as [128,1] cols via row-scan + transpose
        d0row = per.tile([2, 3 * C], F32, tag="d0r", name="d0r")
        nc.vector.memset(d0row[:, 0:C], 1.0 - D0)
        nc.vector.tensor_tensor_scan(d0row[:, C:2 * C], d0row[:, 0:C],
                                     zeros2[0:2, :], 1.0, op0=AL.mult, op1=AL.add)
        nc.vector.memset(d0row[:, 2 * C:2 * C + 1], 1.0)
        nc.vector.tensor_copy(d0row[:, 2 * C + 1:3 * C], d0row[:, C:2 * C - 1])
        pk_ps = ps.tile([128, 8], F32, tag="sm", name="pk")
        nc.tensor.transpose(pk_ps[:, 0:2], d0row[0:2, C:2 * C], ident[0:2, 0:2])
        nc.tensor.transpose(pk_ps[:, 2:4], d0row[0:2, 2 * C:3 * C], ident[0:2, 0:2])
        cstPP = per.tile([128, 2], F32, tag="cstpp", name="cstpp")
        nc.vector.tensor_copy(cstPP[:, 0:1], pk_ps[:, 0:1])   # P0 col
        nc.vector.tensor_copy(cstPP[:, 1:2], pk_ps[:, 2:3])   # Pm10 col
        rPm10 = per.tile([128, 1], F32, tag="rpm0", name="rpm0")
        nc.vector.reciprocal(rPm10[:], cstPP[:, 1:2])
        g1c = 1.1 / (1.0 - D0)

        N2tiles = [per.tile([2, C], F32, tag=f"n2_{i}", name=f"n2_{i}") for i in range(4)]
        carry_ap = n2in_t[:]

        for c in range(NCH):
            c0 = c * C
            KT = [[wk.tile([128, C], F32, tag=f"kt{b}{i}", name=f"kt{b}{i}") for i in range(2)]
                  for b in range(B_LOC)]
            Gsn = [wk.tile([128, C], F32, tag=f"g{b}", name=f"g{b}") for b in range(B_LOC)]
            A = [wk.tile([C, DV], F32, tag=f"a{b}", name=f"a{b}") for b in range(B_LOC)]
            W = [wk.tile([C, DV], F32, tag=f"w{b}", name=f"w{b}") for b in range(B_LOC)]
            R1 = [wk.tile([C, DV], F32, tag=f"r1{b}", name=f"r1{b}") for b in range(B_LOC)]
            etile = [wk.tile([C, DV], F32, tag=f"e{b}", name=f"e{b}") for b in range(B_LOC)]
            utile = [wk.tile([C, DV], F32, tag=f"u{b}", name=f"u{b}") for b in range(B_LOC)]
            sjunk = wk.tile([C, DV], F32, tag="sj", name="sj")
            colsG = [wk.tile([128, 8], F32, tag=f"cg{b}", name=f"cg{b}") for b in range(B_LOC)]
            COLP = wk.tile([128, 6], F32, tag="colp", name="colp")
            ROWP = wk.tile([2, 3 * C], F32, tag="rowp", name="rowp")
            ROWP2 = wk.tile([2, 3 * C], F32, tag="rowp2", name="rowp2")
            COL2 = wk.tile([128, 6], F32, tag="col2", name="col2")

            for b in range(B_LOC):
                KNc = KnN[b][:, c * DK:(c + 1) * DK]
                Vc = V[b][:, c * DV:(c + 1) * DV]
                ktmp = wk.tile([C, DK], F32, tag=f"ktmp{b}", name=f"ktmp{b}")
                nc.sync.dma_start(ktmp[:], keys_d[b, c0:c0 + C, :])
                nc.sync.dma_start(Vc, vals_d[b, c0:c0 + C, :])
                nrm2 = wk.tile([C, 1], F32, tag=f"nn{b}", name=f"nn{b}")
                nc.scalar.activation(sjunk[:], ktmp[:], AF.Square, accum_out=nrm2[:])
                nrm = wk.tile([C, 1], F32, tag=f"nr{b}", name=f"nr{b}")
                nc.scalar.sqrt(nrm[:], nrm2[:])
                nrme = wk.tile([C, 1], F32, tag=f"ne{b}", name=f"ne{b}")
                nc.vector.tensor_scalar_add(nrme[:], nrm[:], EPS)
                rk = wk.tile([C, 1], F32, tag=f"rk{b}", name=f"rk{b}")
                nc.vector.reciprocal(rk[:], nrme[:])
                nc.vector.tensor_scalar_mul(KNc, ktmp[:], rk[:])
                t0 = wk.tile([C, 1], F32, tag=f"t0{b}", name=f"t0{b}")
                nc.vector.tensor_tensor(t0[:], nrm[:], rk[:], op=AL.mult)
                nc.vector.tensor_tensor(knsq[b][:, c:c + 1], t0[:], t0[:], op=AL.mult)
                nc.scalar.activation(sjunk[:], Vc, AF.Square,
                                     accum_out=v2[b][:, c:c + 1])
                if c == 0:
                    for i in range(2):
                        mnat = wk.tile([128, DK], F32, tag=f"mn{b}", name=f"mn{b}")
                        nc.sync.dma_start(mnat[:], mem_d[b, i * 128:(i + 1) * 128, :])
                        for k in range(2):
                            tp = ps2.tile([128, 128], F32, tag="tp", name="tp")
                            nc.tensor.transpose(tp[:], mnat[:, k * 128:(k + 1) * 128],
                                                ident[:])
                            nc.vector.tensor_copy(MT[b][k][:, i * 128:(i + 1) * 128],
                                                  tp[:])
                for k in range(2):
                    tp = ps2.tile([128, 128], F32, tag="tp", name="tp")
                    nc.tensor.transpose(tp[:], KNc[:, k * 128:(k + 1) * 128], ident[:])
                    nc.vector.tensor_copy(KT[b][k][:], tp[:])
                gps = ps.tile([128, C], F32, tag=f"mm{b}", name=f"gps{b}")
                nc.tensor.matmul(gps[:], KT[b][0][:], KT[b][0][:], start=True, stop=False)
                nc.tensor.matmul(gps[:], KT[b][1][:], KT[b][1][:], start=False, stop=True)
                nc.vector.tensor_tensor(Gsn[b][:], gps[:], maskUneg[:], op=AL.mult)
                aps = ps.tile([C, DV], F32, tag=f"mm{b}", name=f"aps{b}")
                nc.tensor.matmul(aps[:], KT[b][0][:], MT[b][0][:], start=True, stop=False)
                nc.tensor.matmul(aps[:], KT[b][1][:], MT[b][1][:], start=False, stop=True)
                nc.vector.tensor_copy(A[b][:], aps[:])
                nc.vector.memset(colsG[b][:, 0:1], g1c)
                nc.vector.tensor_scalar_mul(colsG[b][:, 1:2], rPm10[:],
                                            -0.1 / (1.0 - D0))
                nc.vector.tensor_copy(colsG[b][:, 2:4], cstPP[:, 0:2])

            if use_mhat:
                mh_ps = ps.tile([128, 8], F32, tag="sm", name="mhps")
                nc.tensor.transpose(mh_ps[:, 0:1], mhat_t[0:1, c0:c0 + C],
                                    ident[0:1, 0:1])
                rmx = wk.tile([128, 1], F32, tag="rmx", name="rmx")
                nc.vector.tensor_scalar_add(rmx[:], mh_ps[:, 0:1], EPS)
                nc.vector.reciprocal(rmx[:], rmx[:])

            for j in range(NSOLVE):
                for b in range(B_LOC):
                    g1 = colsG[b][:, 0:1]
                    q2n = colsG[b][:, 1:2]
                    t1 = etile[b]
                    nc.vector.tensor_scalar_mul(t1[:], A[b][:], g1)
                    nc.vector.scalar_tensor_tensor(
                        R1[b][:], V[b][:, c * DV:(c + 1) * DV], q2n, t1[:],
                        op0=AL.mult, op1=AL.add)
                    for it in range(NIT[j]):
                        if j == 0 and it == 0:
                            nc.vector.tensor_copy(W[b][:], R1[b][:])
                            continue
                        sps = ps.tile([C, DV], F32, tag=f"mm{b}", name=f"sps{b}")
                        nc.tensor.matmul(sps[:], Gsn[b][:], W[b][:],
                                         start=True, stop=True)
                        nc.vector.scalar_tensor_tensor(
                            W[b][:], sps[:], g1, R1[b][:], op0=AL.mult, op1=AL.add)
                if j == NSOLVE - 1:
                    break
                # decay update
                for b in range(B_LOC):
                    Pc = colsG[b][:, 2:3]
                    Vc = V[b][:, c * DV:(c + 1) * DV]
                    nc.vector.tensor_scalar_mul(utile[b][:], W[b][:], Pc)
                    nc.vector.tensor_tensor(etile[b][:], utile[b][:], Vc,
                                            op=AL.subtract)
                    nc.scalar.activation(sjunk[:], etile[b][:], AF.Square,
                                         accum_out=colsG[b][:, 6:7],
                                         scale=1.0 / 1.1)
                    nc.scalar.activation(sjunk[:], utile[b][:], AF.Square,
                                         accum_out=colsG[b][:, 5:6])
                    nc.scalar.sqrt(colsG[b][:, 4:5], colsG[b][:, 6:7])
                if use_mhat:
                    rmxc = rmx
                else:
                    mxc = wk.tile([128, 1], F32, tag="mxc", name="mxc")
                    nc.vector.tensor_tensor(mxc[:], colsG[0][:, 4:5],
                                            colsG[1][:, 4:5], op=AL.max)
                    nc.vector.tensor_scalar_add(mxc[:], mxc[:], EPS)
                    rmxc = wk.tile([128, 1], F32, tag="rmxc", name="rmxc")
                    nc.vector.reciprocal(rmxc[:], mxc[:])
                for b in range(B_LOC):
                    u2 = colsG[b][:, 5:6]
                    s2 = colsG[b][:, 6:7]
                    sc = colsG[b][:, 7:8]
                    nc.vector.tensor_scalar(sc, s2, -0.605, None, op0=AL.mult)
                    nc.vector.scalar_tensor_tensor(sc, v2[b][:, c:c + 1], 0.5, sc,
                                                   op0=AL.mult, op1=AL.add)
                    nc.vector.scalar_tensor_tensor(sc, u2, 0.5, sc,
                                                   op0=AL.mult, op1=AL.add)
                    t5 = wk.tile([128, 1], F32, tag=f"t5{b}", name=f"t5{b}")
                    nc.vector.tensor_scalar_mul(t5[:], u2, 1.0 / 1.1)
                    nc.vector.scalar_tensor_tensor(sc, sc, 0.1 / 1.1, t5[:],
                                                   op0=AL.mult, op1=AL.add)
                    omd = wk.tile([128, 1], F32, tag=f"omd{b}", name=f"omd{b}")
                    nc.vector.reciprocal(omd[:], colsG[b][:, 0:1])
                    nc.vector.tensor_tensor(sc, sc, omd[:], op=AL.mult)
                    t6 = wk.tile([128, 1], F32, tag=f"t6{b}", name=f"t6{b}")
                    nc.vector.tensor_tensor(t6[:], u2, knsq[b][:, c:c + 1],
                                            op=AL.mult)
                    nc.vector.scalar_tensor_tensor(COLP[:, 2 + b:3 + b], sc, -2.2,
                                                   t6[:], op0=AL.mult, op1=AL.add)
                    nc.vector.tensor_tensor(t5[:], omd[:], omd[:], op=AL.mult)
                    nc.vector.tensor_scalar_mul(COLP[:, 0 + b:1 + b], t5[:], 1.21)
                    nc.vector.tensor_tensor(COLP[:, 4 + b:5 + b], colsG[b][:, 4:5],
                                            rmxc[:], op=AL.mult)
                    if not use_mhat and j == NSOLVE - 2:
                        nc.vector.tensor_copy(snall[b][:, c:c + 1], colsG[b][:, 4:5])
                tps = ps2.tile([128, 3 * C], F32, tag="tp", name="tps")
                for q in range(3):
                    nc.tensor.transpose(tps[0:2, q * C:(q + 1) * C],
                                        COLP[:, 2 * q:2 * q + 2], ident[:])
                nc.vector.tensor_copy(ROWP[0:2, :], tps[0:2, 0:3 * C])
                n2cur = N2tiles[(c % 2) * 2 + j]
                nc.vector.tensor_tensor_scan(n2cur[:], ROWP[:, 0:C], ROWP[:, C:2 * C],
                                             carry_ap, op0=AL.mult, op1=AL.add)
                utr = wk.tile([2, 2 * C], F32, tag="utr", name="utr")
                nc.vector.tensor_scalar_max(utr[:, 0:C], n2cur[:], 0.0)
                nc.scalar.activation(utr[:, C:2 * C], utr[:, 0:C], AF.Sqrt,
                                     scale=1.0 / (MAXN_EPS * MAXN_EPS))
                nc.vector.tensor_scalar_min(utr[:, 0:C], utr[:, C:2 * C], 1.0)
                drow = wk.tile([2, C], F32, tag="drow", name="drow")
                nc.vector.tensor_scalar(drow[:, :], utr[:, 0:C], 0.001, 0.01,
                                        op0=AL.mult, op1=AL.add)
                nc.vector.scalar_tensor_tensor(drow[:, :], ROWP[:, 2 * C:3 * C], 0.001,
                                               drow[:, :], op0=AL.mult, op1=AL.add)
                nc.vector.tensor_scalar(ROWP2[:, 0:C], drow[:, :], -1.0, 1.0,
                                        op0=AL.mult, op1=AL.add)
                nc.vector.tensor_tensor_scan(ROWP2[:, C:2 * C], ROWP2[:, 0:C],
                                             zeros2[0:2, :], 1.0,
                                             op0=AL.mult, op1=AL.add)
                nc.vector.memset(ROWP2[:, 2 * C:2 * C + 1], 1.0)
                nc.vector.tensor_copy(ROWP2[:, 2 * C + 1:3 * C], ROWP2[:, C:2 * C - 1])
                tps2 = ps.tile([128, 8], F32, tag="sm", name="tps2")
                for q in range(3):
                    nc.tensor.transpose(tps2[:, 2 * q:2 * q + 2],
                                        ROWP2[0:2, q * C:(q + 1) * C], ident[0:2, 0:2])
                nc.vector.tensor_copy(COL2[:, 0:6], tps2[:, 0:6])
                for b in range(B_LOC):
                    omdc = COL2[:, 0 + b:1 + b]
                    nc.vector.reciprocal(colsG[b][:, 7:8], omdc)
                    nc.vector.tensor_scalar_mul(colsG[b][:, 0:1], colsG[b][:, 7:8],
                                                1.1)
                    nc.vector.tensor_copy(colsG[b][:, 2:3], COL2[:, 2 + b:3 + b])
                    nc.vector.tensor_copy(colsG[b][:, 3:4], COL2[:, 4 + b:5 + b])
                    rpm = wk.tile([128, 1], F32, tag=f"rpm{b}", name=f"rpm{b}")
                    nc.vector.reciprocal(rpm[:], COL2[:, 4 + b:5 + b])
                    nc.vector.tensor_tensor(rpm[:], rpm[:], colsG[b][:, 7:8],
                                            op=AL.mult)
                    nc.vector.tensor_scalar_mul(colsG[b][:, 1:2], rpm[:], -0.1)
                if j == NSOLVE - 2:
                    carry_next = n2cur[:, C - 1:C]
            carry_ap = carry_next

            # state update
            for b in range(B_LOC):
                bps = ps.tile([128, 8], F32, tag="sm", name="bps")
                nc.tensor.matmul(bps[:, 0:1], sel127[:], colsG[b][:, 2:3],
                                 start=True, stop=True)
                PCc = wk.tile([128, 1], F32, tag=f"pcc{b}", name=f"pcc{b}")
                nc.vector.tensor_copy(PCc[:], bps[:, 0:1])
                Wn = etile[b]
                nc.vector.tensor_scalar_mul(Wn[:], W[b][:], -1.0)
                KNc = KnN[b][:, c * DK:(c + 1) * DK]
                for i in range(2):
                    mps = ps.tile([128, DV], F32, tag=f"mm{b}", name=f"mps{b}")
                    nc.tensor.matmul(mps[:], KNc[:, i * 128:(i + 1) * 128], Wn[:],
                                     start=True, stop=False)
                    nc.tensor.matmul(mps[:], ident[:], MT[b][i][:],
                                     start=False, stop=True)
                    nc.vector.tensor_scalar_mul(MT[b][i][:], mps[:], PCc[:])

        for b in range(B_LOC):
            if not use_mhat:
                nc.sync.dma_start(nrm_d[b, :, :], snall[b][:])
            for i in range(2):
                st = per.tile([128, DK], F32, tag=f"st{b}{i}", name=f"st{b}{i}")
                for k in range(2):
                    tp = ps2.tile([128, 128], F32, tag="tp", name="tp")
                    nc.tensor.transpose(tp[:], MT[b][k][:, i * 128:(i + 1) * 128],
                                        ident[:])
                    nc.vector.tensor_copy(st[:, k * 128:(k + 1) * 128], tp[:])
                nc.sync.dma_start(out_d[b, i * 128:(i + 1) * 128, :], st[:])
    return nc


def _build(use_mhat):
    key = ("nc", use_mhat)
    if key not in _cache:
        nc = bacc.Bacc("TRN2", target_bir_lowering=False, debug=False, num_devices=8)
        _emit(nc, use_mhat)
        nc.compile()
        _cache[key] = nc
    return _cache[key]


def kernel(memory, keys, values):
    memory = np.ascontiguousarray(memory, np.float32)
    keys = np.ascontiguousarray(keys, np.float32)
    values = np.ascontiguousarray(values, np.float32)
    B = memory.shape[0]
    n2 = (memory.astype(np.float64) ** 2).sum(axis=(1, 2)).astype(np.float32)

    def in_maps(mhat):
        maps = []
        for ci in range(8):
            sl = slice(ci * B_LOC, (ci + 1) * B_LOC)
            maps.append({
                "keys": np.ascontiguousarray(keys[sl]),
                "vals": np.ascontiguousarray(values[sl]),
                "mem": np.ascontiguousarray(memory[sl]),
                "n2in": np.ascontiguousarray(n2[sl].reshape(B_LOC, 1)),
                "mhat": mhat,
            })
        return maps

    zero_mhat = np.zeros((1, S), np.float32)
    nc1 = _build(False)
    r1 = run_bass_kernel_spmd(nc1, in_maps(zero_mhat), core_ids=list(range(8)))
    allnorms = np.concatenate([r["nrm"] for r in r1.results], axis=0)
    norms_t = allnorms.transpose(0, 2, 1).reshape(B, S)
    mhat = np.ascontiguousarray(norms_t.max(axis=0).reshape(1, S).astype(np.float32))

    nc2 = _build(True)
    r2 = run_bass_kernel_spmd(nc2, in_maps(mhat), core_ids=list(range(8)))
    out = np.concatenate([r["out"] for r in r2.results], axis=0)
    return out


# revision 10
# speedup vs baseline: 2.5102x; 1.8152x over previous
"""DynamicDecayMemory Trainium2 kernel (single-launch, 8-core SPMD).

Full inputs: memory (16,256,256), keys (16,4096,256), values (16,4096,256).
Data-parallel over batch: 8 cores x 2 batches each. The sequential scan is
reformulated as chunked (C=128) triangular solves in "w-space"
(u_t = P_t * w_t, P = cumprod(1-d)) solved by Neumann iteration with the
kn-Gram matrix; decay d_t recovered via a small fixed point. The global
cross-batch max of surprise norms: phase 1 runs the scan with the local
2-batch max and records per-step local maxima; an on-device AllReduce(max)
(16KB) produces the global per-step max; phase 2 re-runs the scan with it.
Validated ~3e-6 rel err vs the exact reference.
"""
import sys
import numpy as np

sys.path.insert(0, "/opt/trn_rl_repo")

import concourse.bass as bass
import concourse.bacc as bacc
import concourse.mybir as mybir
import concourse.tile as tile
from concourse import masks
from concourse.bass_utils import run_bass_kernel_spmd
from contextlib import ExitStack

F32 = mybir.dt.float32
AL = mybir.AluOpType
AF = mybir.ActivationFunctionType

B_LOC = 2
S = 4096
C = 128
NCH = S // C
DK = 256
DV = 256
EPS = 1e-6
MAXN_EPS = 256.0 + EPS
D0 = 0.0108

_cache = {}


def _emit(nc):
    keys_d = nc.dram_tensor("keys", [B_LOC, S, DK], F32, kind="ExternalInput")
    vals_d = nc.dram_tensor("vals", [B_LOC, S, DV], F32, kind="ExternalInput")
    mem_d = nc.dram_tensor("mem", [B_LOC, DV, DK], F32, kind="ExternalInput")
    n2in_d = nc.dram_tensor("n2in", [B_LOC, 1], F32, kind="ExternalInput")
    out_d = nc.dram_tensor("out", [B_LOC, DV, DK], F32, kind="ExternalOutput")

    with tile.TileContext(nc) as tc, ExitStack() as ctx:
        per = ctx.enter_context(tc.tile_pool(name="per", bufs=1))
        wk = ctx.enter_context(tc.tile_pool(name="wk", bufs=2))
        ps = ctx.enter_context(tc.tile_pool(name="ps", bufs=1, space="PSUM"))
        ps2 = ctx.enter_context(tc.tile_pool(name="ps2", bufs=2, space="PSUM"))
        dr = ctx.enter_context(tc.tile_pool(name="dram", bufs=1, space="DRAM"))

        KnN = [per.tile([C, NCH * DK], F32, tag=f"kn{b}", name=f"kn{b}")
               for b in range(B_LOC)]
        V = [per.tile([C, NCH * DV], F32, tag=f"v{b}", name=f"v{b}")
             for b in range(B_LOC)]
        MT = [[per.tile([128, DV], F32, tag=f"mt{b}{i}", name=f"mt{b}{i}")
               for i in range(2)] for b in range(B_LOC)]
        knsq = [per.tile([C, NCH], F32, tag=f"ksq{b}", name=f"ksq{b}")
                for b in range(B_LOC)]
        v2 = [per.tile([C, NCH], F32, tag=f"v2{b}", name=f"v2{b}")
              for b in range(B_LOC)]
        mxall = per.tile([C, NCH], F32, tag="mxall", name="mxall")
        mhgrid = per.tile([C, NCH], F32, tag="mhg", name="mhg")

        ident = per.tile([128, 128], F32, tag="ident", name="ident")
        masks.make_identity(nc, ident[:])
        maskUneg = per.tile([128, 128], F32, tag="msku", name="msku")
        masks.make_upper_triangular(nc, maskUneg[:], val=-1.0, diag=False)
        sel127 = per.tile([128, 128], F32, tag="sel127", name="sel127")
        nc.gpsimd.memset(sel127[:], 0.0)
        nc.gpsimd.affine_select(out=sel127[:], in_=sel127[:],
                                compare_op=AL.not_equal, fill=1.0, base=-127,
                                pattern=[[0, 128]], channel_multiplier=1)
        absps = ps2.tile([128, 128], F32, tag="tp", name="absps")
        nc.tensor.transpose(absps[:], ident[:], ident[:])

        zeros2 = per.tile([8, C], F32, tag="zr", name="zr")
        nc.vector.memset(zeros2[:], 0.0)
        n2in_t = per.tile([B_LOC, 1], F32, tag="n2in", name="n2in")
        nc.sync.dma_start(n2in_t[:], n2in_d[:])

        d0row = per.tile([2, 3 * C], F32, tag="d0r", name="d0r")
        nc.vector.memset(d0row[:, 0:C], 1.0 - D0)
        nc.vector.tensor_tensor_scan(d0row[:, C:2 * C], d0row[:, 0:C],
                                     zeros2[0:2, :], 1.0, op0=AL.mult, op1=AL.add)
        nc.vector.memset(d0row[:, 2 * C:2 * C + 1], 1.0)
        nc.vector.tensor_copy(d0row[:, 2 * C + 1:3 * C], d0row[:, C:2 * C - 1])
        pk_ps = ps.tile([128, 8], F32, tag="sm", name="pk")
        nc.tensor.transpose(pk_ps[:, 0:2], d0row[0:2, C:2 * C], ident[0:2, 0:2])
        nc.tensor.transpose(pk_ps[:, 2:4], d0row[0:2, 2 * C:3 * C], ident[0:2, 0:2])
        cstPP = per.tile([128, 2], F32, tag="cstpp", name="cstpp")
        nc.vector.tensor_copy(cstPP[:, 0:1], pk_ps[:, 0:1])
        nc.vector.tensor_copy(cstPP[:, 1:2], pk_ps[:, 2:3])
        rPm10 = per.tile([128, 1], F32, tag="rpm0", name="rpm0")
        nc.vector.reciprocal(rPm10[:], cstPP[:, 1:2])
        g1c = 1.1 / (1.0 - D0)

        N2tiles = [per.tile([2, C], F32, tag=f"n2_{i}", name=f"n2_{i}")
                   for i in range(4)]

        def emit_phase(phase):
            """phase 0: local max, record mxall; phase 1: use mhgrid."""
            NSOLVE = 2 if phase == 0 else 3
            NIT = [3, 2] if phase == 0 else [5, 4, 4]
            carry_ap = n2in_t[:]
            for c in range(NCH):
                c0 = c * C
                KT = [[wk.tile([128, C], F32, tag=f"kt{b}{i}", name=f"kt{b}{i}")
                       for i in range(2)] for b in range(B_LOC)]
                Gsn = [wk.tile([128, C], F32, tag=f"g{b}", name=f"g{b}")
                       for b in range(B_LOC)]
                A = [wk.tile([C, DV], F32, tag=f"a{b}", name=f"a{b}")
                     for b in range(B_LOC)]
                W = [wk.tile([C, DV], F32, tag=f"w{b}", name=f"w{b}")
                     for b in range(B_LOC)]
                R1 = [wk.tile([C, DV], F32, tag=f"r1{b}", name=f"r1{b}")
                      for b in range(B_LOC)]
                etile = [wk.tile([C, DV], F32, tag=f"e{b}", name=f"e{b}")
                         for b in range(B_LOC)]
                utile = [wk.tile([C, DV], F32, tag=f"u{b}", name=f"u{b}")
                         for b in range(B_LOC)]
                sjunk = wk.tile([C, DV], F32, tag="sj", name="sj")
                colsG = [wk.tile([128, 8], F32, tag=f"cg{b}", name=f"cg{b}")
                         for b in range(B_LOC)]
                COLP = wk.tile([128, 6], F32, tag="colp", name="colp")
                ROWP = wk.tile([2, 3 * C], F32, tag="rowp", name="rowp")
                ROWP2 = wk.tile([2, 3 * C], F32, tag="rowp2", name="rowp2")
                COL2 = wk.tile([128, 6], F32, tag="col2", name="col2")

                for b in range(B_LOC):
                    KNc = KnN[b][:, c * DK:(c + 1) * DK]
                    Vc = V[b][:, c * DV:(c + 1) * DV]
                    if phase == 0:
                        ktmp = wk.tile([C, DK], F32, tag=f"ktmp{b}", name=f"ktmp{b}")
                        nc.sync.dma_start(ktmp[:], keys_d[b, c0:c0 + C, :])
                        nc.sync.dma_start(Vc, vals_d[b, c0:c0 + C, :])
                        nrm2 = wk.tile([C, 1], F32, tag=f"nn{b}", name=f"nn{b}")
                        nc.scalar.activation(sjunk[:], ktmp[:], AF.Square,
                                             accum_out=nrm2[:])
                        nrm = wk.tile([C, 1], F32, tag=f"nr{b}", name=f"nr{b}")
                        nc.scalar.sqrt(nrm[:], nrm2[:])
                        nrme = wk.tile([C, 1], F32, tag=f"ne{b}", name=f"ne{b}")
                        nc.vector.tensor_scalar_add(nrme[:], nrm[:], EPS)
                        rk = wk.tile([C, 1], F32, tag=f"rk{b}", name=f"rk{b}")
                        nc.vector.reciprocal(rk[:], nrme[:])
                        nc.vector.tensor_scalar_mul(KNc, ktmp[:], rk[:])
                        t0 = wk.tile([C, 1], F32, tag=f"t0{b}", name=f"t0{b}")
                        nc.vector.tensor_tensor(t0[:], nrm[:], rk[:], op=AL.mult)
                        nc.vector.tensor_tensor(knsq[b][:, c:c + 1], t0[:], t0[:],
                                                op=AL.mult)
                        nc.scalar.activation(sjunk[:], Vc, AF.Square,
                                             accum_out=v2[b][:, c:c + 1])
                    if c == 0:
                        for i in range(2):
                            mnat = wk.tile([128, DK], F32, tag=f"mn{b}", name=f"mn{b}")
                            nc.sync.dma_start(mnat[:], mem_d[b, i * 128:(i + 1) * 128, :])
                            for k in range(2):
                                tp = ps2.tile([128, 128], F32, tag="tp", name="tp")
                                nc.tensor.transpose(tp[:],
                                                    mnat[:, k * 128:(k + 1) * 128],
                                                    ident[:])
                                nc.vector.tensor_copy(
                                    MT[b][k][:, i * 128:(i + 1) * 128], tp[:])
                    for k in range(2):
                        tp = ps2.tile([128, 128], F32, tag="tp", name="tp")
                        nc.tensor.transpose(tp[:], KNc[:, k * 128:(k + 1) * 128],
                                            ident[:])
                        nc.vector.tensor_copy(KT[b][k][:], tp[:])
                    gps = ps.tile([128, C], F32, tag=f"mm{b}", name=f"gps{b}")
                    nc.tensor.matmul(gps[:], KT[b][0][:], KT[b][0][:],
                                     start=True, stop=False)
                    nc.tensor.matmul(gps[:], KT[b][1][:], KT[b][1][:],
                                     start=False, stop=True)
                    nc.vector.tensor_tensor(Gsn[b][:], gps[:], maskUneg[:], op=AL.mult)
                    aps = ps.tile([C, DV], F32, tag=f"mm{b}", name=f"aps{b}")
                    nc.tensor.matmul(aps[:], KT[b][0][:], MT[b][0][:],
                                     start=True, stop=False)
                    nc.tensor.matmul(aps[:], KT[b][1][:], MT[b][1][:],
                                     start=False, stop=True)
                    nc.vector.tensor_copy(A[b][:], aps[:])
                    nc.vector.memset(colsG[b][:, 0:1], g1c)
                    nc.vector.tensor_scalar_mul(colsG[b][:, 1:2], rPm10[:],
                                                -0.1 / (1.0 - D0))
                    nc.vector.tensor_copy(colsG[b][:, 2:4], cstPP[:, 0:2])

                if phase == 1:
                    rmx = wk.tile([128, 1], F32, tag="rmx", name="rmx")
                    nc.vector.tensor_scalar_add(rmx[:], mhgrid[:, c:c + 1], EPS)
                    nc.vector.reciprocal(rmx[:], rmx[:])

                for j in range(NSOLVE):
                    for b in range(B_LOC):
                        g1 = colsG[b][:, 0:1]
                        q2n = colsG[b][:, 1:2]
                        t1 = etile[b]
                        nc.vector.tensor_scalar_mul(t1[:], A[b][:], g1)
                        nc.vector.scalar_tensor_tensor(
                            R1[b][:], V[b][:, c * DV:(c + 1) * DV], q2n, t1[:],
                            op0=AL.mult, op1=AL.add)
                        for it in range(NIT[j]):
                            if j == 0 and it == 0:
                                nc.vector.tensor_copy(W[b][:], R1[b][:])
                                continue
                            sps = ps.tile([C, DV], F32, tag=f"mm{b}", name=f"sps{b}")
                            nc.tensor.matmul(sps[:], Gsn[b][:], W[b][:],
                                             start=True, stop=True)
                            nc.vector.scalar_tensor_tensor(
                                W[b][:], sps[:], g1, R1[b][:], op0=AL.mult, op1=AL.add)
                    if j == NSOLVE - 1:
                        break
                    for b in range(B_LOC):
                        Pc = colsG[b][:, 2:3]
                        Vc = V[b][:, c * DV:(c + 1) * DV]
                        nc.vector.tensor_scalar_mul(utile[b][:], W[b][:], Pc)
                        nc.vector.tensor_tensor(etile[b][:], utile[b][:], Vc,
                                                op=AL.subtract)
                        nc.scalar.activation(sjunk[:], etile[b][:], AF.Square,
                                             accum_out=colsG[b][:, 6:7],
                                             scale=1.0 / 1.1)
                        nc.scalar.activation(sjunk[:], utile[b][:], AF.Square,
                                             accum_out=colsG[b][:, 5:6])
                        nc.scalar.sqrt(colsG[b][:, 4:5], colsG[b][:, 6:7])
                    if phase == 1:
                        rmxc = rmx
                    else:
                        mxc = wk.tile([128, 1], F32, tag="mxc", name="mxc")
                        nc.vector.tensor_tensor(mxc[:], colsG[0][:, 4:5],
                                                colsG[1][:, 4:5], op=AL.max)
                        if j == NSOLVE - 2:
                            nc.vector.tensor_copy(mxall[:, c:c + 1], mxc[:])
                        nc.vector.tensor_scalar_add(mxc[:], mxc[:], EPS)
                        rmxc = wk.tile([128, 1], F32, tag="rmxc", name="rmxc")
                        nc.vector.reciprocal(rmxc[:], mxc[:])
                    for b in range(B_LOC):
                        u2 = colsG[b][:, 5:6]
                        s2 = colsG[b][:, 6:7]
                        sc = colsG[b][:, 7:8]
                        nc.vector.tensor_scalar(sc, s2, -0.605, None, op0=AL.mult)
                        nc.vector.scalar_tensor_tensor(sc, v2[b][:, c:c + 1], 0.5, sc,
                                                       op0=AL.mult, op1=AL.add)
                        nc.vector.scalar_tensor_tensor(sc, u2, 0.5, sc,
                                                       op0=AL.mult, op1=AL.add)
                        t5 = wk.tile([128, 1], F32, tag=f"t5{b}", name=f"t5{b}")
                        nc.vector.tensor_scalar_mul(t5[:], u2, 1.0 / 1.1)
                        nc.vector.scalar_tensor_tensor(sc, sc, 0.1 / 1.1, t5[:],
                                                       op0=AL.mult, op1=AL.add)
                        omd = wk.tile([128, 1], F32, tag=f"omd{b}", name=f"omd{b}")
                        nc.vector.reciprocal(omd[:], colsG[b][:, 0:1])
                        nc.vector.tensor_tensor(sc, sc, omd[:], op=AL.mult)
                        t6 = wk.tile([128, 1], F32, tag=f"t6{b}", name=f"t6{b}")
                        nc.vector.tensor_tensor(t6[:], u2, knsq[b][:, c:c + 1],
                                                op=AL.mult)
                        nc.vector.scalar_tensor_tensor(COLP[:, 2 + b:3 + b], sc, -2.2,
                                                       t6[:], op0=AL.mult, op1=AL.add)
                        nc.vector.tensor_tensor(t5[:], omd[:], omd[:], op=AL.mult)
                        nc.vector.tensor_scalar_mul(COLP[:, 0 + b:1 + b], t5[:], 1.21)
                        nc.vector.tensor_tensor(COLP[:, 4 + b:5 + b], colsG[b][:, 4:5],
                                                rmxc[:], op=AL.mult)
                    tps = ps2.tile([128, 3 * C], F32, tag="tp", name="tps")
                    for q in range(3):
                        nc.tensor.transpose(tps[0:2, q * C:(q + 1) * C],
                                            COLP[:, 2 * q:2 * q + 2], ident[:])
                    nc.vector.tensor_copy(ROWP[0:2, :], tps[0:2, 0:3 * C])
                    n2cur = N2tiles[(c % 2) * 2 + j]
                    nc.vector.tensor_tensor_scan(n2cur[:], ROWP[:, 0:C],
                                                 ROWP[:, C:2 * C], carry_ap,
                                                 op0=AL.mult, op1=AL.add)
                    utr = wk.tile([2, 2 * C], F32, tag="utr", name="utr")
                    nc.vector.tensor_scalar_max(utr[:, 0:C], n2cur[:], 0.0)
                    nc.scalar.activation(utr[:, C:2 * C], utr[:, 0:C], AF.Sqrt,
                                         scale=1.0 / (MAXN_EPS * MAXN_EPS))
                    nc.vector.tensor_scalar_min(utr[:, 0:C], utr[:, C:2 * C], 1.0)
                    drow = wk.tile([2, C], F32, tag="drow", name="drow")
                    nc.vector.tensor_scalar(drow[:, :], utr[:, 0:C], 0.001, 0.01,
                                            op0=AL.mult, op1=AL.add)
                    nc.vector.scalar_tensor_tensor(drow[:, :], ROWP[:, 2 * C:3 * C],
                                                   0.001, drow[:, :],
                                                   op0=AL.mult, op1=AL.add)
                    nc.vector.tensor_scalar(ROWP2[:, 0:C], drow[:, :], -1.0, 1.0,
                                            op0=AL.mult, op1=AL.add)
                    nc.vector.tensor_tensor_scan(ROWP2[:, C:2 * C], ROWP2[:, 0:C],
                                                 zeros2[0:2, :], 1.0,
                                                 op0=AL.mult, op1=AL.add)
                    nc.vector.memset(ROWP2[:, 2 * C:2 * C + 1], 1.0)
                    nc.vector.tensor_copy(ROWP2[:, 2 * C + 1:3 * C],
                                          ROWP2[:, C:2 * C - 1])
                    tps2 = ps.tile([128, 8], F32, tag="sm", name="tps2")
                    for q in range(3):
                        nc.tensor.transpose(tps2[:, 2 * q:2 * q + 2],
                                            ROWP2[0:2, q * C:(q + 1) * C],
                                            ident[0:2, 0:2])
                    nc.vector.tensor_copy(COL2[:, 0:6], tps2[:, 0:6])
                    for b in range(B_LOC):
                        omdc = COL2[:, 0 + b:1 + b]
                        nc.vector.reciprocal(colsG[b][:, 7:8], omdc)
                        nc.vector.tensor_scalar_mul(colsG[b][:, 0:1],
                                                    colsG[b][:, 7:8], 1.1)
                        nc.vector.tensor_copy(colsG[b][:, 2:3], COL2[:, 2 + b:3 + b])
                        nc.vector.tensor_copy(colsG[b][:, 3:4], COL2[:, 4 + b:5 + b])
                        rpm = wk.tile([128, 1], F32, tag=f"rpm{b}", name=f"rpm{b}")
                        nc.vector.reciprocal(rpm[:], COL2[:, 4 + b:5 + b])
                        nc.vector.tensor_tensor(rpm[:], rpm[:], colsG[b][:, 7:8],
                                                op=AL.mult)
                        nc.vector.tensor_scalar_mul(colsG[b][:, 1:2], rpm[:], -0.1)
                    if j == NSOLVE - 2:
                        carry_next = n2cur[:, C - 1:C]
                carry_ap = carry_next

                for b in range(B_LOC):
                    bps = ps.tile([128, 8], F32, tag="sm", name="bps")
                    nc.tensor.matmul(bps[:, 0:1], sel127[:], colsG[b][:, 2:3],
                                     start=True, stop=True)
                    PCc = wk.tile([128, 1], F32, tag=f"pcc{b}", name=f"pcc{b}")
                    nc.vector.tensor_copy(PCc[:], bps[:, 0:1])
                    Wn = etile[b]
                    nc.vector.tensor_scalar_mul(Wn[:], W[b][:], -1.0)
                    KNc = KnN[b][:, c * DK:(c + 1) * DK]
                    for i in range(2):
                        mps = ps.tile([128, DV], F32, tag=f"mm{b}", name=f"mps{b}")
                        nc.tensor.matmul(mps[:], KNc[:, i * 128:(i + 1) * 128], Wn[:],
                                         start=True, stop=False)
                        nc.tensor.matmul(mps[:], ident[:], MT[b][i][:],
                                         start=False, stop=True)
                        nc.vector.tensor_scalar_mul(MT[b][i][:], mps[:], PCc[:])

        emit_phase(0)
        # global per-step max across all 16 batches via AllReduce(max)
        bnc_in = dr.tile([C, NCH], F32, name="bncin")
        bnc_out = dr.tile([C, NCH], F32, name="bncout", addr_space="Shared")
        nc.sync.dma_start(bnc_in[:], mxall[:])
        nc.gpsimd.collective_compute(
            "AllReduce", AL.max,
            ins=[bnc_in.opt()],
            outs=[bnc_out.opt()],
            replica_groups=[list(range(8))],
        )
        nc.sync.dma_start(mhgrid[:], bnc_out[:])
        emit_phase(1)

        for b in range(B_LOC):
            for i in range(2):
                st = per.tile([128, DK], F32, tag=f"st{b}{i}", name=f"st{b}{i}")
                for k in range(2):
                    tp = ps2.tile([128, 128], F32, tag="tp", name="tp")
                    nc.tensor.transpose(tp[:], MT[b][k][:, i * 128:(i + 1) * 128],
                                        ident[:])
                    nc.vector.tensor_copy(st[:, k * 128:(k + 1) * 128], tp[:])
                nc.sync.dma_start(out_d[b, i * 128:(i + 1) * 128, :], st[:])
    return nc


def _build():
    if "nc" not in _cache:
        nc = bacc.Bacc("TRN2", target_bir_lowering=False, debug=False, num_devices=8)
        _emit(nc)
        nc.compile()
        _cache["nc"] = nc
    return _cache["nc"]


def kernel(memory, keys, values):
    memory = np.ascontiguousarray(memory, np.float32)
    keys = np.ascontiguousarray(keys, np.float32)
    values = np.ascontiguousarray(values, np.float32)
    n2 = (memory.astype(np.float64) ** 2).sum(axis=(1, 2)).astype(np.float32)

    maps = []
    for ci in range(8):
        sl = slice(ci * B_LOC, (ci + 1) * B_LOC)
        maps.append({
            "keys": np.ascontiguousarray(keys[sl]),
            "vals": np.ascontiguousarray(values[sl]),
            "mem": np.ascontiguousarray(memory[sl]),
            "n2in": np.ascontiguousarray(n2[sl].reshape(B_LOC, 1)),
        })

    nc = _build()
    r = run_bass_kernel_spmd(nc, maps, core_ids=list(range(8)))
    return np.concatenate([x["out"] for x in r.results], axis=0)


# revision 11
# speedup vs baseline: 2.7361x; 1.0900x over previous
"""DynamicDecayMemory Trainium2 kernel (single-launch, 8-core SPMD).

Full inputs: memory (16,256,256), keys (16,4096,256), values (16,4096,256).
Data-parallel over batch: 8 cores x 2 batches each. The sequential scan is
reformulated as chunked (C=128) triangular solves in "w-space"
(u_t = P_t * w_t, P = cumprod(1-d)) solved by Neumann iteration with the
kn-Gram matrix; decay d_t recovered via a small fixed point. The global
cross-batch max of surprise norms: phase 1 runs the scan with the local
2-batch max and records per-step local maxima; an on-device AllReduce(max)
(16KB) produces the global per-step max; phase 2 re-runs the scan with it.
Validated ~3e-6 rel err vs the exact reference.
"""
import sys
import numpy as np

sys.path.insert(0, "/opt/trn_rl_repo")

import concourse.bass as bass
import concourse.bacc as bacc
import concourse.mybir as mybir
import concourse.tile as tile
from concourse import masks
from concourse.bass_utils import run_bass_kernel_spmd
from contextlib import ExitStack

F32 = mybir.dt.float32
AL = mybir.AluOpType
AF = mybir.ActivationFunctionType

B_LOC = 2
S = 4096
C = 128
NCH = S // C
DK = 256
DV = 256
EPS = 1e-6
MAXN_EPS = 256.0 + EPS
D0 = 0.0108

_cache = {}


def _emit(nc):
    keys_d = nc.dram_tensor("keys", [B_LOC, S, DK], F32, kind="ExternalInput")
    vals_d = nc.dram_tensor("vals", [B_LOC, S, DV], F32, kind="ExternalInput")
    mem_d = nc.dram_tensor("mem", [B_LOC, DV, DK], F32, kind="ExternalInput")
    n2in_d = nc.dram_tensor("n2in", [B_LOC, 1], F32, kind="ExternalInput")
    out_d = nc.dram_tensor("out", [B_LOC, DV, DK], F32, kind="ExternalOutput")

    with tile.TileContext(nc) as tc, ExitStack() as ctx:
        per = ctx.enter_context(tc.tile_pool(name="per", bufs=1))
        wk = ctx.enter_context(tc.tile_pool(name="wk", bufs=2))
        ps = ctx.enter_context(tc.tile_pool(name="ps", bufs=1, space="PSUM"))
        ps2 = ctx.enter_context(tc.tile_pool(name="ps2", bufs=2, space="PSUM"))
        dr = ctx.enter_context(tc.tile_pool(name="dram", bufs=1, space="DRAM"))

        KnN = [per.tile([C, NCH * DK], F32, tag=f"kn{b}", name=f"kn{b}")
               for b in range(B_LOC)]
        V = [per.tile([C, NCH * DV], F32, tag=f"v{b}", name=f"v{b}")
             for b in range(B_LOC)]
        MT = [[per.tile([128, DV], F32, tag=f"mt{b}{i}", name=f"mt{b}{i}")
               for i in range(2)] for b in range(B_LOC)]
        knsq = [per.tile([C, NCH], F32, tag=f"ksq{b}", name=f"ksq{b}")
                for b in range(B_LOC)]
        v2 = [per.tile([C, NCH], F32, tag=f"v2{b}", name=f"v2{b}")
              for b in range(B_LOC)]
        mxall = per.tile([C, NCH], F32, tag="mxall", name="mxall")
        mhgrid = per.tile([C, NCH], F32, tag="mhg", name="mhg")

        ident = per.tile([128, 128], F32, tag="ident", name="ident")
        masks.make_identity(nc, ident[:])
        maskUneg = per.tile([128, 128], F32, tag="msku", name="msku")
        masks.make_upper_triangular(nc, maskUneg[:], val=-1.0, diag=False)
        sel127 = per.tile([128, 128], F32, tag="sel127", name="sel127")
        nc.gpsimd.memset(sel127[:], 0.0)
        nc.gpsimd.affine_select(out=sel127[:], in_=sel127[:],
                                compare_op=AL.not_equal, fill=1.0, base=-127,
                                pattern=[[0, 128]], channel_multiplier=1)
        absps = ps2.tile([128, 128], F32, tag="tp", name="absps")
        nc.tensor.transpose(absps[:], ident[:], ident[:])

        zeros2 = per.tile([8, C], F32, tag="zr", name="zr")
        nc.vector.memset(zeros2[:], 0.0)
        n2in_t = per.tile([B_LOC, 1], F32, tag="n2in", name="n2in")
        nc.sync.dma_start(n2in_t[:], n2in_d[:])

        d0row = per.tile([2, 3 * C], F32, tag="d0r", name="d0r")
        nc.vector.memset(d0row[:, 0:C], 1.0 - D0)
        nc.vector.tensor_tensor_scan(d0row[:, C:2 * C], d0row[:, 0:C],
                                     zeros2[0:2, :], 1.0, op0=AL.mult, op1=AL.add)
        nc.vector.memset(d0row[:, 2 * C:2 * C + 1], 1.0)
        nc.vector.tensor_copy(d0row[:, 2 * C + 1:3 * C], d0row[:, C:2 * C - 1])
        pk_ps = ps.tile([128, 8], F32, tag="sm", name="pk")
        nc.tensor.transpose(pk_ps[:, 0:2], d0row[0:2, C:2 * C], ident[0:2, 0:2])
        nc.tensor.transpose(pk_ps[:, 2:4], d0row[0:2, 2 * C:3 * C], ident[0:2, 0:2])
        cstPP = per.tile([128, 2], F32, tag="cstpp", name="cstpp")
        nc.vector.tensor_copy(cstPP[:, 0:1], pk_ps[:, 0:1])
        nc.vector.tensor_copy(cstPP[:, 1:2], pk_ps[:, 2:3])
        rPm10 = per.tile([128, 1], F32, tag="rpm0", name="rpm0")
        nc.vector.reciprocal(rPm10[:], cstPP[:, 1:2])
        g1c = 1.1 / (1.0 - D0)

        N2tiles = [per.tile([2, C], F32, tag=f"n2_{i}", name=f"n2_{i}")
                   for i in range(4)]

        def emit_phase(phase):
            """phase 0: local max, record mxall; phase 1: use mhgrid."""
            NSOLVE = 2 if phase == 0 else 3
            NIT = [3, 2] if phase == 0 else [4, 3, 3]
            carry_ap = n2in_t[:]
            for c in range(NCH):
                c0 = c * C
                KT = [[wk.tile([128, C], F32, tag=f"kt{b}{i}", name=f"kt{b}{i}")
                       for i in range(2)] for b in range(B_LOC)]
                Gsn = [wk.tile([128, C], F32, tag=f"g{b}", name=f"g{b}")
                       for b in range(B_LOC)]
                A = [wk.tile([C, DV], F32, tag=f"a{b}", name=f"a{b}")
                     for b in range(B_LOC)]
                W = [wk.tile([C, DV], F32, tag=f"w{b}", name=f"w{b}")
                     for b in range(B_LOC)]
                R1 = [wk.tile([C, DV], F32, tag=f"r1{b}", name=f"r1{b}")
                      for b in range(B_LOC)]
                etile = [wk.tile([C, DV], F32, tag=f"e{b}", name=f"e{b}")
                         for b in range(B_LOC)]
                utile = [wk.tile([C, DV], F32, tag=f"u{b}", name=f"u{b}")
                         for b in range(B_LOC)]
                sjunk = wk.tile([C, DV], F32, tag="sj", name="sj")
                colsG = [wk.tile([128, 8], F32, tag=f"cg{b}", name=f"cg{b}")
                         for b in range(B_LOC)]
                COLP = wk.tile([128, 6], F32, tag="colp", name="colp")
                ROWP = wk.tile([2, 3 * C], F32, tag="rowp", name="rowp")
                ROWP2 = wk.tile([2, 3 * C], F32, tag="rowp2", name="rowp2")
                COL2 = wk.tile([128, 6], F32, tag="col2", name="col2")

                for b in range(B_LOC):
                    KNc = KnN[b][:, c * DK:(c + 1) * DK]
                    Vc = V[b][:, c * DV:(c + 1) * DV]
                    if phase == 0:
                        ktmp = wk.tile([C, DK], F32, tag=f"ktmp{b}", name=f"ktmp{b}")
                        nc.sync.dma_start(ktmp[:], keys_d[b, c0:c0 + C, :])
                        nc.sync.dma_start(Vc, vals_d[b, c0:c0 + C, :])
                        nrm2 = wk.tile([C, 1], F32, tag=f"nn{b}", name=f"nn{b}")
                        nc.scalar.activation(sjunk[:], ktmp[:], AF.Square,
                                             accum_out=nrm2[:])
                        nrm = wk.tile([C, 1], F32, tag=f"nr{b}", name=f"nr{b}")
                        nc.scalar.sqrt(nrm[:], nrm2[:])
                        nrme = wk.tile([C, 1], F32, tag=f"ne{b}", name=f"ne{b}")
                        nc.vector.tensor_scalar_add(nrme[:], nrm[:], EPS)
                        rk = wk.tile([C, 1], F32, tag=f"rk{b}", name=f"rk{b}")
                        nc.vector.reciprocal(rk[:], nrme[:])
                        nc.vector.tensor_scalar_mul(KNc, ktmp[:], rk[:])
                        t0 = wk.tile([C, 1], F32, tag=f"t0{b}", name=f"t0{b}")
                        nc.vector.tensor_tensor(t0[:], nrm[:], rk[:], op=AL.mult)
                        nc.vector.tensor_tensor(knsq[b][:, c:c + 1], t0[:], t0[:],
                                                op=AL.mult)
                        nc.scalar.activation(sjunk[:], Vc, AF.Square,
                                             accum_out=v2[b][:, c:c + 1])
                    if c == 0:
                        for i in range(2):
                            mnat = wk.tile([128, DK], F32, tag=f"mn{b}", name=f"mn{b}")
                            nc.sync.dma_start(mnat[:], mem_d[b, i * 128:(i + 1) * 128, :])
                            for k in range(2):
                                tp = ps2.tile([128, 128], F32, tag="tp", name="tp")
                                nc.tensor.transpose(tp[:],
                                                    mnat[:, k * 128:(k + 1) * 128],
                                                    ident[:])
                                nc.vector.tensor_copy(
                                    MT[b][k][:, i * 128:(i + 1) * 128], tp[:])
                    for k in range(2):
                        tp = ps2.tile([128, 128], F32, tag="tp", name="tp")
                        nc.tensor.transpose(tp[:], KNc[:, k * 128:(k + 1) * 128],
                                            ident[:])
                        nc.scalar.copy(KT[b][k][:], tp[:])
                    gps = ps.tile([128, C], F32, tag=f"mm{b}", name=f"gps{b}")
                    nc.tensor.matmul(gps[:], KT[b][0][:], KT[b][0][:],
                                     start=True, stop=False)
                    nc.tensor.matmul(gps[:], KT[b][1][:], KT[b][1][:],
                                     start=False, stop=True)
                    nc.vector.tensor_tensor(Gsn[b][:], gps[:], maskUneg[:], op=AL.mult)
                    aps = ps.tile([C, DV], F32, tag=f"mm{b}", name=f"aps{b}")
                    nc.tensor.matmul(aps[:], KT[b][0][:], MT[b][0][:],
                                     start=True, stop=False)
                    nc.tensor.matmul(aps[:], KT[b][1][:], MT[b][1][:],
                                     start=False, stop=True)
                    nc.scalar.copy(A[b][:], aps[:])
                    nc.vector.memset(colsG[b][:, 0:1], g1c)
                    nc.vector.tensor_scalar_mul(colsG[b][:, 1:2], rPm10[:],
                                                -0.1 / (1.0 - D0))
                    nc.vector.tensor_copy(colsG[b][:, 2:4], cstPP[:, 0:2])

                if phase == 1:
                    rmx = wk.tile([128, 1], F32, tag="rmx", name="rmx")
                    nc.vector.tensor_scalar_add(rmx[:], mhgrid[:, c:c + 1], EPS)
                    nc.vector.reciprocal(rmx[:], rmx[:])

                for j in range(NSOLVE):
                    for b in range(B_LOC):
                        g1 = colsG[b][:, 0:1]
                        q2n = colsG[b][:, 1:2]
                        t1 = etile[b]
                        nc.vector.tensor_scalar_mul(t1[:], A[b][:], g1)
                        nc.vector.scalar_tensor_tensor(
                            R1[b][:], V[b][:, c * DV:(c + 1) * DV], q2n, t1[:],
                            op0=AL.mult, op1=AL.add)
                        for it in range(NIT[j]):
                            if j == 0 and it == 0:
                                nc.vector.tensor_copy(W[b][:], R1[b][:])
                                continue
                            sps = ps.tile([C, DV], F32, tag=f"mm{b}", name=f"sps{b}")
                            nc.tensor.matmul(sps[:], Gsn[b][:], W[b][:],
                                             start=True, stop=True)
                            nc.vector.scalar_tensor_tensor(
                                W[b][:], sps[:], g1, R1[b][:], op0=AL.mult, op1=AL.add)
                    if j == NSOLVE - 1:
                        break
                    for b in range(B_LOC):
                        Pc = colsG[b][:, 2:3]
                        Vc = V[b][:, c * DV:(c + 1) * DV]
                        nc.vector.tensor_scalar_mul(utile[b][:], W[b][:], Pc)
                        nc.vector.tensor_tensor(etile[b][:], utile[b][:], Vc,
                                                op=AL.subtract)
                        nc.scalar.activation(sjunk[:], etile[b][:], AF.Square,
                                             accum_out=colsG[b][:, 6:7],
                                             scale=1.0 / 1.1)
                        nc.scalar.activation(sjunk[:], utile[b][:], AF.Square,
                                             accum_out=colsG[b][:, 5:6])
                        nc.scalar.sqrt(colsG[b][:, 4:5], colsG[b][:, 6:7])
                    if phase == 1:
                        rmxc = rmx
                    else:
                        mxc = wk.tile([128, 1], F32, tag="mxc", name="mxc")
                        nc.vector.tensor_tensor(mxc[:], colsG[0][:, 4:5],
                                                colsG[1][:, 4:5], op=AL.max)
                        if j == NSOLVE - 2:
                            nc.vector.tensor_copy(mxall[:, c:c + 1], mxc[:])
                        nc.vector.tensor_scalar_add(mxc[:], mxc[:], EPS)
                        rmxc = wk.tile([128, 1], F32, tag="rmxc", name="rmxc")
                        nc.vector.reciprocal(rmxc[:], mxc[:])
                    for b in range(B_LOC):
                        u2 = colsG[b][:, 5:6]
                        s2 = colsG[b][:, 6:7]
                        sc = colsG[b][:, 7:8]
                        nc.vector.tensor_scalar(sc, s2, -0.605, None, op0=AL.mult)
                        nc.vector.scalar_tensor_tensor(sc, v2[b][:, c:c + 1], 0.5, sc,
                                                       op0=AL.mult, op1=AL.add)
                        nc.vector.scalar_tensor_tensor(sc, u2, 0.5, sc,
                                                       op0=AL.mult, op1=AL.add)
                        t5 = wk.tile([128, 1], F32, tag=f"t5{b}", name=f"t5{b}")
                        nc.vector.tensor_scalar_mul(t5[:], u2, 1.0 / 1.1)
                        nc.vector.scalar_tensor_tensor(sc, sc, 0.1 / 1.1, t5[:],
                                                       op0=AL.mult, op1=AL.add)
                        omd = wk.tile([128, 1], F32, tag=f"omd{b}", name=f"omd{b}")
                        nc.vector.reciprocal(omd[:], colsG[b][:, 0:1])
                        nc.vector.tensor_tensor(sc, sc, omd[:], op=AL.mult)
                        t6 = wk.tile([128, 1], F32, tag=f"t6{b}", name=f"t6{b}")
                        nc.vector.tensor_tensor(t6[:], u2, knsq[b][:, c:c + 1],
                                                op=AL.mult)
                        nc.vector.scalar_tensor_tensor(COLP[:, 2 + b:3 + b], sc, -2.2,
                                                       t6[:], op0=AL.mult, op1=AL.add)
                        nc.vector.tensor_tensor(t5[:], omd[:], omd[:], op=AL.mult)
                        nc.vector.tensor_scalar_mul(COLP[:, 0 + b:1 + b], t5[:], 1.21)
                        nc.vector.tensor_tensor(COLP[:, 4 + b:5 + b], colsG[b][:, 4:5],
                                                rmxc[:], op=AL.mult)
                    tps = ps2.tile([128, 3 * C], F32, tag="tp", name="tps")
                    for q in range(3):
                        nc.tensor.transpose(tps[0:2, q * C:(q + 1) * C],
                                            COLP[:, 2 * q:2 * q + 2], ident[:])
                    nc.vector.tensor_copy(ROWP[0:2, :], tps[0:2, 0:3 * C])
                    n2cur = N2tiles[(c % 2) * 2 + j]
                    nc.vector.tensor_tensor_scan(n2cur[:], ROWP[:, 0:C],
                                                 ROWP[:, C:2 * C], carry_ap,
                                                 op0=AL.mult, op1=AL.add)
                    utr = wk.tile([2, 2 * C], F32, tag="utr", name="utr")
                    nc.vector.tensor_scalar_max(utr[:, 0:C], n2cur[:], 0.0)
                    nc.scalar.activation(utr[:, C:2 * C], utr[:, 0:C], AF.Sqrt,
                                         scale=1.0 / (MAXN_EPS * MAXN_EPS))
                    nc.vector.tensor_scalar_min(utr[:, 0:C], utr[:, C:2 * C], 1.0)
                    drow = wk.tile([2, C], F32, tag="drow", name="drow")
                    nc.vector.tensor_scalar(drow[:, :], utr[:, 0:C], 0.001, 0.01,
                                            op0=AL.mult, op1=AL.add)
                    nc.vector.scalar_tensor_tensor(drow[:, :], ROWP[:, 2 * C:3 * C],
                                                   0.001, drow[:, :],
                                                   op0=AL.mult, op1=AL.add)
                    nc.vector.tensor_scalar(ROWP2[:, 0:C], drow[:, :], -1.0, 1.0,
                                            op0=AL.mult, op1=AL.add)
                    nc.vector.tensor_tensor_scan(ROWP2[:, C:2 * C], ROWP2[:, 0:C],
                                                 zeros2[0:2, :], 1.0,
                                                 op0=AL.mult, op1=AL.add)
                    nc.vector.memset(ROWP2[:, 2 * C:2 * C + 1], 1.0)
                    nc.vector.tensor_copy(ROWP2[:, 2 * C + 1:3 * C],
                                          ROWP2[:, C:2 * C - 1])
                    tps2 = ps.tile([128, 8], F32, tag="sm", name="tps2")
                    for q in range(3):
                        nc.tensor.transpose(tps2[:, 2 * q:2 * q + 2],
                                            ROWP2[0:2, q * C:(q + 1) * C],
                                            ident[0:2, 0:2])
                    nc.vector.tensor_copy(COL2[:, 0:6], tps2[:, 0:6])
                    for b in range(B_LOC):
                        omdc = COL2[:, 0 + b:1 + b]
                        nc.vector.reciprocal(colsG[b][:, 7:8], omdc)
                        nc.vector.tensor_scalar_mul(colsG[b][:, 0:1],
                                                    colsG[b][:, 7:8], 1.1)
                        nc.vector.tensor_copy(colsG[b][:, 2:3], COL2[:, 2 + b:3 + b])
                        nc.vector.tensor_copy(colsG[b][:, 3:4], COL2[:, 4 + b:5 + b])
                        rpm = wk.tile([128, 1], F32, tag=f"rpm{b}", name=f"rpm{b}")
                        nc.vector.reciprocal(rpm[:], COL2[:, 4 + b:5 + b])
                        nc.vector.tensor_tensor(rpm[:], rpm[:], colsG[b][:, 7:8],
                                                op=AL.mult)
                        nc.vector.tensor_scalar_mul(colsG[b][:, 1:2], rpm[:], -0.1)
                    if j == NSOLVE - 2:
                        carry_next = n2cur[:, C - 1:C]
                carry_ap = carry_next

                for b in range(B_LOC):
                    bps = ps.tile([128, 8], F32, tag="sm", name="bps")
                    nc.tensor.matmul(bps[:, 0:1], sel127[:], colsG[b][:, 2:3],
                                     start=True, stop=True)
                    PCc = wk.tile([128, 1], F32, tag=f"pcc{b}", name=f"pcc{b}")
                    nc.vector.tensor_copy(PCc[:], bps[:, 0:1])
                    Wn = etile[b]
                    nc.vector.tensor_scalar_mul(Wn[:], W[b][:], -1.0)
                    KNc = KnN[b][:, c * DK:(c + 1) * DK]
                    for i in range(2):
                        mps = ps.tile([128, DV], F32, tag=f"mm{b}", name=f"mps{b}")
                        nc.tensor.matmul(mps[:], KNc[:, i * 128:(i + 1) * 128], Wn[:],
                                         start=True, stop=False)
                        nc.tensor.matmul(mps[:], ident[:], MT[b][i][:],
                                         start=False, stop=True)
                        nc.vector.tensor_scalar_mul(MT[b][i][:], mps[:], PCc[:])

        emit_phase(0)
        # global per-step max across all 16 batches via AllReduce(max)
        bnc_in = dr.tile([C, NCH], F32, name="bncin")
        bnc_out = dr.tile([C, NCH], F32, name="bncout", addr_space="Shared")
        nc.sync.dma_start(bnc_in[:], mxall[:])
        nc.gpsimd.collective_compute(
            "AllReduce", AL.max,
            ins=[bnc_in.opt()],
            outs=[bnc_out.opt()],
            replica_groups=[list(range(8))],
        )
        nc.sync.dma_start(mhgrid[:], bnc_out[:])
        emit_phase(1)

        for b in range(B_LOC):
            for i in range(2):
                st = per.tile([128, DK], F32, tag=f"st{b}{i}", name=f"st{b}{i}")
                for k in range(2):
                    tp = ps2.tile([128, 128], F32, tag="tp", name="tp")
                    nc.tensor.transpose(tp[:], MT[b][k][:, i * 128:(i + 1) * 128],
                                        ident[:])
                    nc.vector.tensor_copy(st[:, k * 128:(k + 1) * 128], tp[:])
                nc.sync.dma_start(out_d[b, i * 128:(i + 1) * 128, :], st[:])
    return nc


def _build():
    if "nc" not in _cache:
        nc = bacc.Bacc("TRN2", target_bir_lowering=False, debug=False, num_devices=8)
        _emit(nc)
        nc.compile()
        _cache["nc"] = nc
    return _cache["nc"]


def kernel(memory, keys, values):
    memory = np.ascontiguousarray(memory, np.float32)
    keys = np.ascontiguousarray(keys, np.float32)
    values = np.ascontiguousarray(values, np.float32)
    n2 = (memory.astype(np.float64) ** 2).sum(axis=(1, 2)).astype(np.float32)

    maps = []
    for ci in range(8):
        sl = slice(ci * B_LOC, (ci + 1) * B_LOC)
        maps.append({
            "keys": np.ascontiguousarray(keys[sl]),
            "vals": np.ascontiguousarray(values[sl]),
            "mem": np.ascontiguousarray(memory[sl]),
            "n2in": np.ascontiguousarray(n2[sl].reshape(B_LOC, 1)),
        })

    nc = _build()
    r = run_bass_kernel_spmd(nc, maps, core_ids=list(range(8)))
    return np.concatenate([x["out"] for x in r.results], axis=0)
